# revision 6
# baseline (speedup 1.0000x reference)
"""Causal self-attention on 8 Trainium2 NeuronCores.

Sharding: core c handles batch b = c//2 and heads [(c%2)*8, (c%2)*8+8).
Each core computes the full QKV projection for its head slice, causal
flash-style attention, and the row-parallel w_o partial product. The two
partials per batch are summed on the host (no device collectives).

All PE matmuls run in fp16 (1 cycle/row) with fp32 PSUM accumulation.
Feature-major layouts throughout:
  x^T [D, N]        (host pre-transposed)
  Q^T, K^T [ch, N]  (from GEMM with W stationary, x^T moving)
  V [N, ch] + ones  (from GEMM with x^T stationary, W moving)
  S^T [k, q] = K^T_tile.T @ Q^T  -> exp -> P^T [k, q]
  O^T [ch, q] = (V|1).T @ P^T    (row 64 = softmax denominator)
  y = O^T_norm.T @ W_o           (accumulated over ch tiles)

Causal masking: diagonal-straddling S^T blocks only compute columns
>= delta (cols below are fully masked); the 128-wide partial strip gets
-BIG * max(k - u, 0) added via an extra accumulating matmul so exp()
underflows to exact zeros.

The attention kt loop is ACT(exp)-bound; PE idle inside it is filled by
interleaving the previous chunk's out-projection and the next chunk's
QKV matmuls as filler quanta.
"""

import numpy as np

B, N, D, H = 4, 2048, 1024, 16
DH = 64
N_CORES = 8
HPC = 8            # heads per core
CH = HPC * DH      # 512 channels per core
SCALE = 1.0 / 8.0  # 1/sqrt(DH)
BIG = 280.0        # SCALE*BIG = 35 >> max |S/8|, exp underflows to 0

_cached = None


def _build_program():
    from contextlib import ExitStack

    import concourse.tile as tile
    from concourse import bacc, mybir

    f16 = mybir.dt.float16
    f32 = mybir.dt.float32
    Exp = mybir.ActivationFunctionType.Exp
    mult = mybir.AluOpType.mult
    add = mybir.AluOpType.add

    nc = bacc.Bacc(
        "TRN2", target_bir_lowering=False, debug=False, num_devices=N_CORES
    )

    xT_d = nc.dram_tensor("xT", [D, N], f16, kind="ExternalInput").ap()
    wq_d = nc.dram_tensor("wq", [D, CH], f16, kind="ExternalInput").ap()
    wk_d = nc.dram_tensor("wk", [D, CH], f16, kind="ExternalInput").ap()
    wv_d = nc.dram_tensor("wv", [D, CH], f16, kind="ExternalInput").ap()
    wo_d = nc.dram_tensor("wo", [CH, D], f16, kind="ExternalInput").ap()
    bq_d = nc.dram_tensor("bq", [CH, 1], f32, kind="ExternalInput").ap()
    bk_d = nc.dram_tensor("bk", [CH, 1], f32, kind="ExternalInput").ap()
    bv_d = nc.dram_tensor("bvb", [128, CH], f32, kind="ExternalInput").ap()
    bo_d = nc.dram_tensor("bob", [128, D], f32, kind="ExternalInput").ap()
    U_d = nc.dram_tensor("U", [128, 128], f16, kind="ExternalInput").ap()
    R_d = nc.dram_tensor("R", [128, 896], f16, kind="ExternalInput").ap()
    y_d = nc.dram_tensor("y", [N, D], f16, kind="ExternalOutput").ap()

    with tile.TileContext(nc) as tc, ExitStack() as ctx:
        const = ctx.enter_context(tc.tile_pool(name="const", bufs=1))
        actp = ctx.enter_context(tc.tile_pool(name="actp", bufs=1))
        work = ctx.enter_context(tc.tile_pool(name="work", bufs=3))
        normp = ctx.enter_context(tc.tile_pool(name="normp", bufs=2))
        ps_s = ctx.enter_context(tc.tile_pool(name="ps_s", bufs=2, space="PSUM"))
        ps_p = ctx.enter_context(tc.tile_pool(name="ps_p", bufs=4, space="PSUM"))

        # ---- constants / weights into SBUF ----
        # K-weights + first seq-chunk of x first so the K^T GEMM starts ASAP.
        wq = [const.tile([128, CH], f16, tag=f"wq{i}", name=f"wq{i}") for i in range(8)]
        wk = [const.tile([128, CH], f16, tag=f"wk{i}", name=f"wk{i}") for i in range(8)]
        wv = [const.tile([128, CH], f16, tag=f"wv{i}", name=f"wv{i}") for i in range(8)]
        xt = [[const.tile([128, 512], f16, tag=f"xt{i}_{sc}", name=f"xt{i}_{sc}")
               for sc in range(4)] for i in range(8)]
        # Round-robin input DMAs across engine queues so the 2D row-descriptor
        # processing runs in parallel; first-needed first. The first wave
        # (wk + first x chunk) additionally uses the vector/scalar queues,
        # which are compute-idle until the first GEMM finishes.
        first_engs = [nc.sync, nc.gpsimd, nc.scalar]
        engs = [nc.sync, nc.gpsimd]
        _ei = [0]

        def dma_first(dst, src):
            first_engs[_ei[0] % len(first_engs)].dma_start(dst, src)
            _ei[0] += 1

        def dma_in(dst, src):
            engs[_ei[0] % len(engs)].dma_start(dst, src)
            _ei[0] += 1

        for i in range(8):
            dma_first(wk[i][:], wk_d[i * 128 : (i + 1) * 128, :])
            dma_first(xt[i][0][:], xT_d[i * 128 : (i + 1) * 128, 0:512])
        bq = [const.tile([128, 1], f32, tag=f"bq{j}", name=f"bq{j}") for j in range(4)]
        bk = [const.tile([128, 1], f32, tag=f"bk{j}", name=f"bk{j}") for j in range(4)]
        for j in range(4):
            dma_first(bq[j][:], bq_d[j * 128 : (j + 1) * 128, :])
            dma_first(bk[j][:], bk_d[j * 128 : (j + 1) * 128, :])
        bv_t = const.tile([128, CH], f32, tag="bvb", name="bvb")
        dma_first(bv_t[:], bv_d[:])
        for i in range(8):
            dma_first(wv[i][:], wv_d[i * 128 : (i + 1) * 128, :])
            dma_first(wq[i][:], wq_d[i * 128 : (i + 1) * 128, :])
        for sc in range(1, 4):
            for i in range(8):
                dma_in(xt[i][sc][:],
                       xT_d[i * 128 : (i + 1) * 128, sc * 512 : (sc + 1) * 512])
        U_t = const.tile([128, 128], f16, tag="U", name="Ut")
        dma_in(U_t[:], U_d[:])
        R_t = const.tile([128, 896], f16, tag="R", name="Rt")
        dma_in(R_t[:], R_d[:])
        wo = [const.tile([128, D], f16, tag=f"wo{j}", name=f"wo{j}") for j in range(4)]
        for j in range(4):
            dma_in(wo[j][:], wo_d[j * 128 : (j + 1) * 128, :])
        bo_t = const.tile([128, D], f32, tag="bob", name="bob")
        dma_in(bo_t[:], bo_d[:])

        # ---- persistent activations ----
        QT = [[actp.tile([128, 512], f16, tag=f"qt{ct}_{sc}", name=f"qt{ct}_{sc}") for sc in range(4)]
              for ct in range(4)]
        KT = [[actp.tile([128, 512], f16, tag=f"kt{ct}_{sc}", name=f"kt{ct}_{sc}") for sc in range(4)]
              for ct in range(4)]
        V = [actp.tile([128, 8 * 65], f16, tag=f"v{st}", name=f"v{st}") for st in range(16)]
        OTn = [[actp.tile([128, 512], f16, tag=f"otn{hp}_{qc}", name=f"otn{hp}_{qc}") for qc in range(4)]
               for hp in range(4)]

        # ---- PE filler quanta -------------------------------------------
        # The attention kt loop leaves the PE ~40% idle (waiting on ACT exp).
        # Those slots are filled by popping emission closures from a queue:
        # the previous chunk's out-projection groups and the next chunk's
        # QKV GEMM groups.
        filler = []

        def pop_filler():
            if filler:
                filler.pop(0)()

        def drain_filler():
            while filler:
                filler.pop(0)()

        def emit_outproj_group(qc, stl, oc):
            st = 4 * qc + stl
            sl = slice(stl * 128, (stl + 1) * 128)
            ocs = slice(oc * 512, (oc + 1) * 512)
            yp = ps_p.tile([128, 512], f32, tag="p512", name="p512")
            for hpp in range(4):
                nc.tensor.matmul(yp[:], OTn[hpp][qc][:, sl],
                                 wo[hpp][:, ocs],
                                 start=(hpp == 0), stop=(hpp == 3),
                                 skip_group_check=True)
            ysb = work.tile([128, 512], f16, tag="ysb", name="ysb")
            nc.vector.scalar_tensor_tensor(ysb[:], yp[:], 1.0,
                                           bo_t[:, ocs], mult, add)
            nc.sync.dma_start(y_d[st * 128 : (st + 1) * 128, ocs], ysb[:])

        def queue_outproj(qc):
            for stl in range(4):
                for oc in range(2):
                    filler.append(
                        lambda qc=qc, stl=stl, oc=oc: emit_outproj_group(qc, stl, oc))

        def emit_kt_group(ct, sc):
            cs = slice(ct * 128, (ct + 1) * 128)
            p = ps_p.tile([128, 512], f32, tag="p512", name="p512")
            for d in range(8):
                nc.tensor.matmul(p[:], wk[d][:, cs], xt[d][sc][:],
                                 start=(d == 0), stop=(d == 7),
                                 skip_group_check=True)
            nc.vector.tensor_scalar_add(KT[ct][sc][:], p[:], bk[ct][:])

        def emit_qt_group(ct, sc):
            cs = slice(ct * 128, (ct + 1) * 128)
            p = ps_p.tile([128, 512], f32, tag="p512", name="p512")
            for d in range(8):
                nc.tensor.matmul(p[:], wq[d][:, cs], xt[d][sc][:],
                                 start=(d == 0), stop=(d == 7),
                                 skip_group_check=True)
            nc.vector.tensor_scalar_add(QT[ct][sc][:], p[:], bq[ct][:])

        def emit_v_group(stl, sc):
            st = 4 * sc + stl
            ts = slice(stl * 128, (stl + 1) * 128)
            p = ps_p.tile([128, 512], f32, tag="p512", name="p512")
            for d in range(8):
                nc.tensor.matmul(p[:], xt[d][sc][:, ts], wv[d][:, :],
                                 start=(d == 0), stop=(d == 7),
                                 skip_group_check=True)
            v3 = V[st][:].rearrange("p (h e) -> p h e", e=65)
            nc.vector.scalar_tensor_tensor(
                v3[:, :, 0:64],
                p[:].rearrange("p (h e) -> p h e", e=64),
                1.0,
                bv_t[:].rearrange("p (h e) -> p h e", e=64),
                mult, add,
            )
            nc.vector.memset(v3[:, :, 64:65], 1.0)

        def queue_qkv(sc):
            # K first (S-matmul stationary), then V, then Q.
            for ct in range(4):
                filler.append(lambda ct=ct, sc=sc: emit_kt_group(ct, sc))
            for stl in range(4):
                filler.append(lambda stl=stl, sc=sc: emit_v_group(stl, sc))
            for ct in range(4):
                filler.append(lambda ct=ct, sc=sc: emit_qt_group(ct, sc))

        # ---- chunk 0 QKV runs up front (nothing to hide it under) ----
        queue_qkv(0)
        drain_filler()

        for sc in range(4):
            # Queue this chunk's deferred PE work: previous chunk's
            # out-projection, then the NEXT chunk's QKV projections.
            if sc > 0:
                queue_outproj(sc - 1)
            if sc < 3:
                queue_qkv(sc + 1)

            # attention for query chunk qc = sc, head pairs interleaved so the
            # even head's K=64 matmuls (rows 0-63) and the odd head's (rows
            # 64-127) run concurrently in the PE array.
            qc = sc
            nkt = 4 * (qc + 1)
            for hp in range(4):
                h0, h1 = 2 * hp, 2 * hp + 1
                av0 = ps_p.tile([65, 512], f32, tag="p512", name="av0")
                av1 = ps_p.tile([65, 512], f32, tag="p512", name="av1")

                # software pipeline: S(kt) -> exp(kt) on ACT while PE runs
                # S(kt+1); AV(kt) issues after S(kt+1) so PE never waits exp.
                pend = []  # (kt, pt tile, delta) awaiting AV

                def emit_av(kt, pt, delta, first, last):
                    nc.tensor.matmul(
                        av0[:, delta:512], V[kt][:, h0 * 65 : h0 * 65 + 65],
                        pt[:, delta:512],
                        start=first, stop=last, skip_group_check=True)
                    nc.tensor.matmul(
                        av1[:, delta:512], V[kt][:, h1 * 65 : h1 * 65 + 65],
                        pt[:, 512 + delta : 1024],
                        start=first, stop=last, skip_group_check=True)

                for kt in range(nkt):
                    # merged S^T tile: cols 0-511 head h0, 512-1023 head h1.
                    # Diagonal-straddling blocks: cols < delta are fully
                    # masked -> not computed at all; the 128-wide strip
                    # [delta, delta+128) gets the -BIG ramp added.
                    diag = kt >= 4 * qc
                    delta = 128 * kt - 512 * qc if diag else 0
                    sp = ps_s.tile([128, 1024], f32, tag="s2", name="sp")
                    kcol = slice((kt % 4) * 128, (kt % 4) * 128 + 128)
                    nc.tensor.matmul(
                        sp[:, delta:512], KT[hp][kt // 4][0:64, kcol],
                        QT[hp][qc][0:64, delta:512],
                        start=True, stop=not diag, skip_group_check=True)
                    nc.tensor.matmul(
                        sp[:, 512 + delta : 1024], KT[hp][kt // 4][64:128, kcol],
                        QT[hp][qc][64:128, delta:512],
                        start=True, stop=not diag, skip_group_check=True)
                    if diag:
                        rsl = slice(384, 512)
                        nc.tensor.matmul(sp[:, delta : delta + 128], U_t[:],
                                         R_t[:, rsl],
                                         start=False, stop=True,
                                         skip_group_check=True)
                        nc.tensor.matmul(sp[:, 512 + delta : 512 + delta + 128],
                                         U_t[:], R_t[:, rsl],
                                         start=False, stop=True,
                                         skip_group_check=True)
                    pt = work.tile([128, 1024], f16, tag="pt", name="pt")
                    sp3 = sp[:].rearrange("p (h q) -> p h q", h=2)
                    pt3 = pt[:].rearrange("p (h q) -> p h q", h=2)
                    nc.scalar.activation(pt3[:, :, delta:512],
                                         sp3[:, :, delta:512], Exp, scale=SCALE)
                    pend.append((kt, pt, delta))
                    pop_filler()
                    if len(pend) > 1:
                        k0, p0, d0 = pend.pop(0)
                        emit_av(k0, p0, d0, k0 == 0, False)
                k0, p0, d0 = pend.pop(0)
                emit_av(k0, p0, d0, k0 == 0, True)

                # softmax normalization: evacuate av PSUM to SBUF right away
                # (frees the PSUM pool), then normalize from SBUF off the
                # critical path. Row 64 of av = softmax denominator.
                avsb = normp.tile([65, 1024], f32, tag="avsb", name="avsb")
                nc.vector.tensor_copy(avsb[:, 0:512], av0[:])
                nc.vector.tensor_copy(avsb[:, 512:1024], av1[:])
                r0 = work.tile([1, 512], f32, tag="r", name="r0")
                nc.vector.reciprocal(r0[:], avsb[64:65, 0:512])
                rb0 = work.tile([64, 512], f32, tag="rb", name="rb0")
                nc.gpsimd.partition_broadcast(rb0[:], r0[:], channels=64)
                nc.vector.tensor_mul(OTn[hp][qc][0:64, :], avsb[0:64, 0:512],
                                     rb0[:])
                r1 = work.tile([1, 512], f32, tag="r", name="r1")
                nc.vector.reciprocal(r1[:], avsb[64:65, 512:1024])
                rb1 = work.tile([64, 512], f32, tag="rb", name="rb1")
                nc.gpsimd.partition_broadcast(rb1[:], r1[:], channels=64)
                nc.vector.tensor_mul(OTn[hp][qc][64:128, :], avsb[0:64, 512:1024],
                                     rb1[:])
            drain_filler()
        queue_outproj(3)
        drain_filler()

    nc.compile()
    return nc


def _host_inputs(x, w_qkv, b_qkv, w_o, b_o):
    """Per-core input dicts implementing the sharding + layout prep."""
    U = np.zeros((128, 128), np.float16)
    for c in range(128):
        U[c, c:] = 1.0
    R = np.zeros((128, 896), np.float16)
    for c in range(128):
        R[c, : c + 384] = -BIG

    in_maps = []
    for c in range(N_CORES):
        b = c // 2
        hs = (c % 2) * HPC
        cols = slice(hs * DH, (hs + HPC) * DH)
        in_maps.append({
            "xT": np.ascontiguousarray(x[b].T).astype(np.float16),
            "wq": w_qkv[:, cols].astype(np.float16),
            "wk": w_qkv[:, D:][:, cols].astype(np.float16),
            "wv": w_qkv[:, 2 * D:][:, cols].astype(np.float16),
            "wo": w_o[hs * DH : (hs + HPC) * DH, :].astype(np.float16),
            "bq": b_qkv[cols].reshape(CH, 1).astype(np.float32),
            "bk": b_qkv[D:][cols].reshape(CH, 1).astype(np.float32),
            "bvb": np.tile(b_qkv[2 * D:][cols].astype(np.float32), (128, 1)),
            "bob": np.tile(b_o.astype(np.float32), (128, 1)),
            "U": U,
            "R": R,
        })
    return in_maps


def kernel(x, w_qkv, b_qkv, w_o, b_o):
    global _cached
    from concourse.bass_utils import run_bass_kernel_spmd

    x = np.asarray(x)
    w_qkv = np.asarray(w_qkv)
    b_qkv = np.asarray(b_qkv)
    w_o = np.asarray(w_o)
    b_o = np.asarray(b_o)

    if _cached is None:
        _cached = _build_program()
    nc = _cached

    in_maps = _host_inputs(x, w_qkv, b_qkv, w_o, b_o)
    res = run_bass_kernel_spmd(nc, in_maps, list(range(N_CORES)))

    out = np.empty((B, N, D), np.float32)
    for b in range(B):
        out[b] = (res.results[2 * b]["y"].astype(np.float32)
                  + res.results[2 * b + 1]["y"].astype(np.float32))
    return out


# revision 12
# speedup vs baseline: 1.0861x; 1.0861x over previous
"""Causal self-attention on 8 Trainium2 NeuronCores.

Sharding: core c handles batch b = c//2 and heads [(c%2)*8, (c%2)*8+8).
Each core computes the full QKV projection for its head slice, causal
flash-style attention, and the row-parallel w_o partial product. The two
partials per batch are summed on the host (no device collectives).

All PE matmuls run in fp16 (1 cycle/row) with fp32 PSUM accumulation.
Feature-major layouts throughout:
  x^T [D, N]        (host pre-transposed)
  Q^T, K^T [ch, N]  (from GEMM with W stationary, x^T moving)
  V [N, ch] + ones  (from GEMM with x^T stationary, W moving)
  S^T [k, q] = K^T_tile.T @ Q^T  -> exp -> P^T [k, q]
  O^T [ch, q] = (V|1).T @ P^T    (row 64 = softmax denominator)
  y = O^T_norm.T @ W_o           (accumulated over ch tiles)

Causal masking: diagonal-straddling S^T blocks only compute columns
>= delta (cols below are fully masked); the 128-wide partial strip gets
-BIG * max(k - u, 0) added via an extra accumulating matmul so exp()
underflows to exact zeros.

The attention kt loop is ACT(exp)-bound; PE idle inside it is filled by
interleaving the previous chunk's out-projection and the next chunk's
QKV matmuls as filler quanta.
"""

import numpy as np

B, N, D, H = 4, 2048, 1024, 16
DH = 64
N_CORES = 8
HPC = 8            # heads per core
CH = HPC * DH      # 512 channels per core
SCALE = 1.0 / 8.0  # 1/sqrt(DH)
BIG = 280.0        # SCALE*BIG = 35 >> max |S/8|, exp underflows to 0

_cached = None


def _build_program():
    from contextlib import ExitStack

    import concourse.tile as tile
    from concourse import bacc, mybir

    f16 = mybir.dt.float16
    f32 = mybir.dt.float32
    Exp = mybir.ActivationFunctionType.Exp
    Ln = mybir.ActivationFunctionType.Ln
    mult = mybir.AluOpType.mult
    add = mybir.AluOpType.add

    nc = bacc.Bacc(
        "TRN2", target_bir_lowering=False, debug=False, num_devices=N_CORES
    )

    xT_d = nc.dram_tensor("xT", [D, N], f16, kind="ExternalInput").ap()
    wq_d = nc.dram_tensor("wq", [D, CH], f16, kind="ExternalInput").ap()
    wk_d = nc.dram_tensor("wk", [D, CH], f16, kind="ExternalInput").ap()
    wv_d = nc.dram_tensor("wv", [D, CH], f16, kind="ExternalInput").ap()
    wo_d = nc.dram_tensor("wo", [CH, D], f16, kind="ExternalInput").ap()
    bq_d = nc.dram_tensor("bq", [CH, 1], f32, kind="ExternalInput").ap()
    bk_d = nc.dram_tensor("bk", [CH, 1], f32, kind="ExternalInput").ap()
    bv_d = nc.dram_tensor("bvb", [128, CH], f32, kind="ExternalInput").ap()
    bo_d = nc.dram_tensor("bob", [128, D], f32, kind="ExternalInput").ap()
    U_d = nc.dram_tensor("U", [128, 128], f16, kind="ExternalInput").ap()
    R_d = nc.dram_tensor("R", [128, 896], f16, kind="ExternalInput").ap()
    y_d = nc.dram_tensor("y", [N, D], f16, kind="ExternalOutput").ap()

    with tile.TileContext(nc) as tc, ExitStack() as ctx:
        const = ctx.enter_context(tc.tile_pool(name="const", bufs=1))
        actp = ctx.enter_context(tc.tile_pool(name="actp", bufs=1))
        work = ctx.enter_context(tc.tile_pool(name="work", bufs=3))
        ptp = ctx.enter_context(tc.tile_pool(name="ptp", bufs=5))
        normp = ctx.enter_context(tc.tile_pool(name="normp", bufs=2))
        ps_s = ctx.enter_context(tc.tile_pool(name="ps_s", bufs=2, space="PSUM"))
        ps_p = ctx.enter_context(tc.tile_pool(name="ps_p", bufs=4, space="PSUM"))

        # ---- constants / weights into SBUF ----
        # K-weights + first seq-chunk of x first so the K^T GEMM starts ASAP.
        wq = [const.tile([128, CH], f16, tag=f"wq{i}", name=f"wq{i}") for i in range(8)]
        wk = [const.tile([128, CH], f16, tag=f"wk{i}", name=f"wk{i}") for i in range(8)]
        wv = [const.tile([128, CH], f16, tag=f"wv{i}", name=f"wv{i}") for i in range(8)]
        xt = [[const.tile([128, 512], f16, tag=f"xt{i}_{sc}", name=f"xt{i}_{sc}")
               for sc in range(4)] for i in range(8)]
        # Round-robin input DMAs across engine queues so the 2D row-descriptor
        # processing runs in parallel; first-needed first. The first wave
        # (wk + first x chunk) additionally uses the vector/scalar queues,
        # which are compute-idle until the first GEMM finishes.
        first_engs = [nc.sync, nc.gpsimd, nc.scalar]
        engs = [nc.sync, nc.gpsimd]
        _ei = [0]

        def dma_first(dst, src):
            first_engs[_ei[0] % len(first_engs)].dma_start(dst, src)
            _ei[0] += 1

        def dma_in(dst, src):
            engs[_ei[0] % len(engs)].dma_start(dst, src)
            _ei[0] += 1

        # wave 1: just the ct=0 column slices of wk/wq + x chunk 0 + wv,
        # so the first KT/QT/V GEMMs can start after ~2.5MB instead of ~5MB.
        for i in range(8):
            dma_first(wk[i][:, 0:128], wk_d[i * 128 : (i + 1) * 128, 0:128])
            dma_first(xt[i][0][:], xT_d[i * 128 : (i + 1) * 128, 0:512])
        bq = [const.tile([128, 1], f32, tag=f"bq{j}", name=f"bq{j}") for j in range(4)]
        bk = [const.tile([128, 1], f32, tag=f"bk{j}", name=f"bk{j}") for j in range(4)]
        dma_first(bk[0][:], bk_d[0:128, :])
        dma_first(bq[0][:], bq_d[0:128, :])
        bv_t = const.tile([128, CH], f32, tag="bvb", name="bvb")
        dma_first(bv_t[:], bv_d[:])
        for i in range(8):
            dma_first(wq[i][:, 0:128], wq_d[i * 128 : (i + 1) * 128, 0:128])
            dma_first(wv[i][:], wv_d[i * 128 : (i + 1) * 128, :])
        # wave 2: remainders and later chunks.
        for i in range(8):
            dma_in(wk[i][:, 128:CH], wk_d[i * 128 : (i + 1) * 128, 128:CH])
            dma_in(wq[i][:, 128:CH], wq_d[i * 128 : (i + 1) * 128, 128:CH])
        for j in range(1, 4):
            dma_in(bq[j][:], bq_d[j * 128 : (j + 1) * 128, :])
            dma_in(bk[j][:], bk_d[j * 128 : (j + 1) * 128, :])
        for sc in range(1, 4):
            for i in range(8):
                dma_in(xt[i][sc][:],
                       xT_d[i * 128 : (i + 1) * 128, sc * 512 : (sc + 1) * 512])
        U_t = const.tile([128, 128], f16, tag="U", name="Ut")
        dma_in(U_t[:], U_d[:])
        R_t = const.tile([128, 896], f16, tag="R", name="Rt")
        dma_in(R_t[:], R_d[:])
        wo = [const.tile([128, D], f16, tag=f"wo{j}", name=f"wo{j}") for j in range(4)]
        for j in range(4):
            dma_in(wo[j][:], wo_d[j * 128 : (j + 1) * 128, :])
        bo_t = const.tile([128, D], f32, tag="bob", name="bob")
        dma_in(bo_t[:], bo_d[:])

        # ---- persistent activations ----
        QT = [[actp.tile([128, 512], f16, tag=f"qt{ct}_{sc}", name=f"qt{ct}_{sc}") for sc in range(4)]
              for ct in range(4)]
        KT = [[actp.tile([128, 512], f16, tag=f"kt{ct}_{sc}", name=f"kt{ct}_{sc}") for sc in range(4)]
              for ct in range(4)]
        V = [actp.tile([128, 8 * 65], f16, tag=f"v{st}", name=f"v{st}") for st in range(16)]
        OTn = [[actp.tile([128, 512], f16, tag=f"otn{hp}_{qc}", name=f"otn{hp}_{qc}") for qc in range(4)]
               for hp in range(4)]

        # ---- PE filler quanta -------------------------------------------
        # The attention kt loop leaves the PE ~40% idle (waiting on ACT exp).
        # Those slots are filled by popping emission closures from a queue:
        # the previous chunk's out-projection groups and the next chunk's
        # QKV GEMM groups.
        filler = []

        def pop_filler():
            if filler:
                filler.pop(0)()

        def drain_filler():
            while filler:
                filler.pop(0)()

        def emit_outproj_group(qc, stl, oc):
            st = 4 * qc + stl
            sl = slice(stl * 128, (stl + 1) * 128)
            ocs = slice(oc * 512, (oc + 1) * 512)
            yp = ps_p.tile([128, 512], f32, tag="p512", name="p512")
            for hpp in range(4):
                nc.tensor.matmul(yp[:], OTn[hpp][qc][:, sl],
                                 wo[hpp][:, ocs],
                                 start=(hpp == 0), stop=(hpp == 3),
                                 skip_group_check=True)
            ysb = work.tile([128, 512], f16, tag="ysb", name="ysb")
            nc.vector.scalar_tensor_tensor(ysb[:], yp[:], 1.0,
                                           bo_t[:, ocs], mult, add)
            nc.sync.dma_start(y_d[st * 128 : (st + 1) * 128, ocs], ysb[:])

        def queue_outproj(qc):
            for stl in range(4):
                for oc in range(2):
                    filler.append(
                        lambda qc=qc, stl=stl, oc=oc: emit_outproj_group(qc, stl, oc))

        def emit_kt_group(ct, sc):
            cs = slice(ct * 128, (ct + 1) * 128)
            p = ps_p.tile([128, 512], f32, tag="p512", name="p512")
            for d in range(8):
                nc.tensor.matmul(p[:], wk[d][:, cs], xt[d][sc][:],
                                 start=(d == 0), stop=(d == 7),
                                 skip_group_check=True)
            nc.vector.tensor_scalar_add(KT[ct][sc][:], p[:], bk[ct][:])

        def emit_qt_group(ct, sc):
            cs = slice(ct * 128, (ct + 1) * 128)
            p = ps_p.tile([128, 512], f32, tag="p512", name="p512")
            for d in range(8):
                nc.tensor.matmul(p[:], wq[d][:, cs], xt[d][sc][:],
                                 start=(d == 0), stop=(d == 7),
                                 skip_group_check=True)
            nc.vector.tensor_scalar_add(QT[ct][sc][:], p[:], bq[ct][:])

        def emit_v_group(stl, sc):
            st = 4 * sc + stl
            ts = slice(stl * 128, (stl + 1) * 128)
            p = ps_p.tile([128, 512], f32, tag="p512", name="p512")
            for d in range(8):
                nc.tensor.matmul(p[:], xt[d][sc][:, ts], wv[d][:, :],
                                 start=(d == 0), stop=(d == 7),
                                 skip_group_check=True)
            v3 = V[st][:].rearrange("p (h e) -> p h e", e=65)
            nc.vector.scalar_tensor_tensor(
                v3[:, :, 0:64],
                p[:].rearrange("p (h e) -> p h e", e=64),
                1.0,
                bv_t[:].rearrange("p (h e) -> p h e", e=64),
                mult, add,
            )
            nc.vector.memset(v3[:, :, 64:65], 1.0)

        def queue_qkv(sc):
            # K first (S-matmul stationary), then V, then Q.
            for ct in range(4):
                filler.append(lambda ct=ct, sc=sc: emit_kt_group(ct, sc))
            for stl in range(4):
                filler.append(lambda stl=stl, sc=sc: emit_v_group(stl, sc))
            for ct in range(4):
                filler.append(lambda ct=ct, sc=sc: emit_qt_group(ct, sc))

        # ---- chunk 0 QKV: only what hp0's attention needs runs up front
        # (KT/QT ct=0 + all V); the other head-pairs' K/Q projections are
        # queued as filler consumed during earlier head-pairs' kt loops.
        emit_kt_group(0, 0)
        emit_qt_group(0, 0)
        for stl in range(4):
            emit_v_group(stl, 0)

        for sc in range(4):
            # Queue this chunk's deferred PE work: previous chunk's
            # out-projection, then the NEXT chunk's QKV projections.
            if sc == 0:
                for ct in range(1, 4):
                    filler.append(lambda ct=ct: emit_kt_group(ct, 0))
                    filler.append(lambda ct=ct: emit_qt_group(ct, 0))
            if sc > 0:
                queue_outproj(sc - 1)
            if sc < 3:
                queue_qkv(sc + 1)

            # attention for query chunk qc = sc, head pairs interleaved so the
            # even head's K=64 matmuls (rows 0-63) and the odd head's (rows
            # 64-127) run concurrently in the PE array.
            qc = sc
            nkt = 4 * (qc + 1)
            for hp in range(4):
                h0, h1 = 2 * hp, 2 * hp + 1
                av0 = ps_p.tile([65, 512], f32, tag="p512", name="av0")
                av1 = ps_p.tile([65, 512], f32, tag="p512", name="av1")

                # software pipeline: S(kt) -> exp(kt) on ACT while PE runs
                # S(kt+1); AV(kt) issues after S(kt+1) so PE never waits exp.
                pend = []  # (kt, pt tile, delta) awaiting AV

                def emit_av(kt, pt, delta, first, last):
                    nc.tensor.matmul(
                        av0[:, delta:512], V[kt][:, h0 * 65 : h0 * 65 + 65],
                        pt[:, delta:512],
                        start=first, stop=last, skip_group_check=True)
                    nc.tensor.matmul(
                        av1[:, delta:512], V[kt][:, h1 * 65 : h1 * 65 + 65],
                        pt[:, 512 + delta : 1024],
                        start=first, stop=last, skip_group_check=True)

                for kt in range(nkt):
                    # merged S^T tile: cols 0-511 head h0, 512-1023 head h1.
                    # Diagonal-straddling blocks: cols < delta are fully
                    # masked -> not computed at all; the 128-wide strip
                    # [delta, delta+128) gets the -BIG ramp added.
                    diag = kt >= 4 * qc
                    delta = 128 * kt - 512 * qc if diag else 0
                    sp = ps_s.tile([128, 1024], f32, tag="s2", name="sp")
                    kcol = slice((kt % 4) * 128, (kt % 4) * 128 + 128)
                    nc.tensor.matmul(
                        sp[:, delta:512], KT[hp][kt // 4][0:64, kcol],
                        QT[hp][qc][0:64, delta:512],
                        start=True, stop=not diag, skip_group_check=True)
                    nc.tensor.matmul(
                        sp[:, 512 + delta : 1024], KT[hp][kt // 4][64:128, kcol],
                        QT[hp][qc][64:128, delta:512],
                        start=True, stop=not diag, skip_group_check=True)
                    if diag:
                        rsl = slice(384, 512)
                        nc.tensor.matmul(sp[:, delta : delta + 128], U_t[:],
                                         R_t[:, rsl],
                                         start=False, stop=True,
                                         skip_group_check=True)
                        nc.tensor.matmul(sp[:, 512 + delta : 512 + delta + 128],
                                         U_t[:], R_t[:, rsl],
                                         start=False, stop=True,
                                         skip_group_check=True)
                    pt = ptp.tile([128, 1024], f16, tag="pt", name="pt")
                    sp3 = sp[:].rearrange("p (h q) -> p h q", h=2)
                    pt3 = pt[:].rearrange("p (h q) -> p h q", h=2)
                    nc.scalar.activation(pt3[:, :, delta:512],
                                         sp3[:, :, delta:512], Exp, scale=SCALE)
                    pend.append((kt, pt, delta))
                    pop_filler()
                    # AVs emitted in 2-kt batches: fewer S<->AV stationary
                    # switches means fewer un-hidden LDWEIGHTS on the PE.
                    if kt % 2 == 1 and len(pend) > 2:
                        for _ in range(2):
                            k0, p0, d0 = pend.pop(0)
                            emit_av(k0, p0, d0, k0 == 0, False)
                while pend:
                    k0, p0, d0 = pend.pop(0)
                    emit_av(k0, p0, d0, k0 == 0, not pend)

                # softmax normalization: evacuate av PSUM to SBUF right away
                # (frees the PSUM pool), then normalize from SBUF off the
                # critical path. Row 64 of av = softmax denominator.
                avsb = normp.tile([65, 1024], f32, tag="avsb", name="avsb")
                nc.vector.tensor_copy(avsb[:, 0:512], av0[:])
                nc.vector.tensor_copy(avsb[:, 512:1024], av1[:])
                r = work.tile([1, 1024], f32, tag="r", name="r")
                if qc < 3 or hp == 3:
                    # 1/d = exp(-ln(d)) on ACT (both fns share one table
                    # set). ACT has slack in the PE-bound chunks and after
                    # the final chunk's last exp; the [1,N] reciprocal on
                    # DVE would throttle the chunk cadence instead.
                    lnt = work.tile([1, 1024], f32, tag="lnt", name="lnt")
                    nc.scalar.activation(lnt[:], avsb[64:65, :], Ln)
                    nc.scalar.activation(r[:], lnt[:], Exp, scale=-1.0)
                else:
                    nc.vector.reciprocal(r[:, 0:512], avsb[64:65, 0:512])
                    nc.vector.reciprocal(r[:, 512:1024], avsb[64:65, 512:1024])
                rb0 = work.tile([64, 512], f32, tag="rb", name="rb0")
                nc.gpsimd.partition_broadcast(rb0[:], r[0:1, 0:512], channels=64)
                nc.vector.tensor_mul(OTn[hp][qc][0:64, :], avsb[0:64, 0:512],
                                     rb0[:])
                rb1 = work.tile([64, 512], f32, tag="rb", name="rb1")
                nc.gpsimd.partition_broadcast(rb1[:], r[0:1, 512:1024], channels=64)
                nc.vector.tensor_mul(OTn[hp][qc][64:128, :], avsb[0:64, 512:1024],
                                     rb1[:])
            drain_filler()
        queue_outproj(3)
        drain_filler()

    nc.compile()
    return nc


def _host_inputs(x, w_qkv, b_qkv, w_o, b_o):
    """Per-core input dicts implementing the sharding + layout prep."""
    U = np.zeros((128, 128), np.float16)
    for c in range(128):
        U[c, c:] = 1.0
    R = np.zeros((128, 896), np.float16)
    for c in range(128):
        R[c, : c + 384] = -BIG

    in_maps = []
    for c in range(N_CORES):
        b = c // 2
        hs = (c % 2) * HPC
        cols = slice(hs * DH, (hs + HPC) * DH)
        in_maps.append({
            "xT": np.ascontiguousarray(x[b].T).astype(np.float16),
            "wq": w_qkv[:, cols].astype(np.float16),
            "wk": w_qkv[:, D:][:, cols].astype(np.float16),
            "wv": w_qkv[:, 2 * D:][:, cols].astype(np.float16),
            "wo": w_o[hs * DH : (hs + HPC) * DH, :].astype(np.float16),
            "bq": b_qkv[cols].reshape(CH, 1).astype(np.float32),
            "bk": b_qkv[D:][cols].reshape(CH, 1).astype(np.float32),
            "bvb": np.tile(b_qkv[2 * D:][cols].astype(np.float32), (128, 1)),
            "bob": np.tile(b_o.astype(np.float32), (128, 1)),
            "U": U,
            "R": R,
        })
    return in_maps


def kernel(x, w_qkv, b_qkv, w_o, b_o):
    global _cached
    from concourse.bass_utils import run_bass_kernel_spmd

    x = np.asarray(x)
    w_qkv = np.asarray(w_qkv)
    b_qkv = np.asarray(b_qkv)
    w_o = np.asarray(w_o)
    b_o = np.asarray(b_o)

    if _cached is None:
        _cached = _build_program()
    nc = _cached

    in_maps = _host_inputs(x, w_qkv, b_qkv, w_o, b_o)
    res = run_bass_kernel_spmd(nc, in_maps, list(range(N_CORES)))

    out = np.empty((B, N, D), np.float32)
    for b in range(B):
        out[b] = (res.results[2 * b]["y"].astype(np.float32)
                  + res.results[2 * b + 1]["y"].astype(np.float32))
    return out


# revision 14
# speedup vs baseline: 1.1718x; 1.0789x over previous
"""Causal self-attention on 8 Trainium2 NeuronCores.

Sharding: core c handles batch b = c//2 and heads [(c%2)*8, (c%2)*8+8).
Each core computes the full QKV projection for its head slice, causal
flash-style attention, and the row-parallel w_o partial product. The two
partials per batch are summed on the host (no device collectives).

All PE matmuls run in fp16 (1 cycle/row) with fp32 PSUM accumulation.
Feature-major layouts throughout:
  x^T [D, N]        (host pre-transposed)
  Q^T, K^T [ch, N]  (from GEMM with W stationary, x^T moving)
  V [N, ch] + ones  (from GEMM with x^T stationary, W moving)
  S^T [k, q] = K^T_tile.T @ Q^T  -> exp -> P^T [k, q]
  O^T [ch, q] = (V|1).T @ P^T    (row 64 = softmax denominator)
  y = O^T_norm.T @ W_o           (accumulated over ch tiles)

Causal masking: diagonal-straddling S^T blocks only compute columns
>= delta (cols below are fully masked); the 128-wide partial strip gets
-BIG * max(k - u, 0) added via an extra accumulating matmul so exp()
underflows to exact zeros.

The attention kt loop is ACT(exp)-bound; PE idle inside it is filled by
interleaving the previous chunk's out-projection and the next chunk's
QKV matmuls as filler quanta.
"""

import numpy as np

B, N, D, H = 4, 2048, 1024, 16
DH = 64
N_CORES = 8
HPC = 8            # heads per core
CH = HPC * DH      # 512 channels per core
SCALE = 1.0 / 8.0  # 1/sqrt(DH)
BIG = 280.0        # SCALE*BIG = 35 >> max |S/8|, exp underflows to 0

_cached = None


def _build_program():
    from contextlib import ExitStack

    import concourse.tile as tile
    from concourse import bacc, mybir

    f16 = mybir.dt.float16
    f32 = mybir.dt.float32
    Exp = mybir.ActivationFunctionType.Exp
    Ln = mybir.ActivationFunctionType.Ln
    mult = mybir.AluOpType.mult
    add = mybir.AluOpType.add

    # The kernel uses both Exp (softmax) and Ln (reciprocal-via-exp(-ln)).
    # The table-load placement pass assigns each activation the first set
    # containing its function, which thrashes ~27 ACT_TABLE_LOADs between
    # `exp_and_others` and `natural_log`. Restrict Exp/Ln to the combined
    # `natural_log_exp_and_others` set (names/order unchanged, so the
    # act_func_set_id indexing stays valid) -> exactly one load.
    orig_tables = bacc.get_activation_tables

    def _patched_tables(arch):
        t = dict(orig_tables(arch))
        for name, fns in t.items():
            if name != "natural_log_exp_and_others":
                t[name] = {
                    f for f in fns
                    if f not in (mybir.ActivationFunctionType.Exp,
                                 mybir.ActivationFunctionType.Ln)
                }
        return t

    bacc.get_activation_tables = _patched_tables

    nc = bacc.Bacc(
        "TRN2", target_bir_lowering=False, debug=False, num_devices=N_CORES
    )

    xT_d = nc.dram_tensor("xT", [D, N], f16, kind="ExternalInput").ap()
    wq_d = nc.dram_tensor("wq", [D, CH], f16, kind="ExternalInput").ap()
    wk_d = nc.dram_tensor("wk", [D, CH], f16, kind="ExternalInput").ap()
    wv_d = nc.dram_tensor("wv", [D, CH], f16, kind="ExternalInput").ap()
    wo_d = nc.dram_tensor("wo", [CH, D], f16, kind="ExternalInput").ap()
    bq_d = nc.dram_tensor("bq", [CH, 1], f32, kind="ExternalInput").ap()
    bk_d = nc.dram_tensor("bk", [CH, 1], f32, kind="ExternalInput").ap()
    bv_d = nc.dram_tensor("bvb", [128, CH], f32, kind="ExternalInput").ap()
    bo_d = nc.dram_tensor("bob", [128, D], f32, kind="ExternalInput").ap()
    U_d = nc.dram_tensor("U", [128, 128], f16, kind="ExternalInput").ap()
    R_d = nc.dram_tensor("R", [128, 896], f16, kind="ExternalInput").ap()
    y_d = nc.dram_tensor("y", [N, D], f16, kind="ExternalOutput").ap()

    with tile.TileContext(nc) as tc, ExitStack() as ctx:
        const = ctx.enter_context(tc.tile_pool(name="const", bufs=1))
        actp = ctx.enter_context(tc.tile_pool(name="actp", bufs=1))
        work = ctx.enter_context(tc.tile_pool(name="work", bufs=3))
        ptp = ctx.enter_context(tc.tile_pool(name="ptp", bufs=5))
        normp = ctx.enter_context(tc.tile_pool(name="normp", bufs=2))
        ps_s = ctx.enter_context(tc.tile_pool(name="ps_s", bufs=2, space="PSUM"))
        ps_p = ctx.enter_context(tc.tile_pool(name="ps_p", bufs=4, space="PSUM"))

        # ---- constants / weights into SBUF ----
        # K-weights + first seq-chunk of x first so the K^T GEMM starts ASAP.
        wq = [const.tile([128, CH], f16, tag=f"wq{i}", name=f"wq{i}") for i in range(8)]
        wk = [const.tile([128, CH], f16, tag=f"wk{i}", name=f"wk{i}") for i in range(8)]
        wv = [const.tile([128, CH], f16, tag=f"wv{i}", name=f"wv{i}") for i in range(8)]
        xt = [[const.tile([128, 512], f16, tag=f"xt{i}_{sc}", name=f"xt{i}_{sc}")
               for sc in range(4)] for i in range(8)]
        # Round-robin input DMAs across engine queues so the 2D row-descriptor
        # processing runs in parallel; first-needed first. The first wave
        # (wk + first x chunk) additionally uses the vector/scalar queues,
        # which are compute-idle until the first GEMM finishes.
        first_engs = [nc.sync, nc.gpsimd, nc.scalar]
        engs = [nc.sync, nc.gpsimd]
        _ei = [0]

        def dma_first(dst, src):
            first_engs[_ei[0] % len(first_engs)].dma_start(dst, src)
            _ei[0] += 1

        def dma_in(dst, src):
            engs[_ei[0] % len(engs)].dma_start(dst, src)
            _ei[0] += 1

        # wave 1: just the ct=0 column slices of wk/wq + x chunk 0 + wv,
        # so the first KT/QT/V GEMMs can start after ~2.5MB instead of ~5MB.
        for i in range(8):
            dma_first(wk[i][:, 0:128], wk_d[i * 128 : (i + 1) * 128, 0:128])
            dma_first(xt[i][0][:], xT_d[i * 128 : (i + 1) * 128, 0:512])
        bq = [const.tile([128, 1], f32, tag=f"bq{j}", name=f"bq{j}") for j in range(4)]
        bk = [const.tile([128, 1], f32, tag=f"bk{j}", name=f"bk{j}") for j in range(4)]
        dma_first(bk[0][:], bk_d[0:128, :])
        dma_first(bq[0][:], bq_d[0:128, :])
        bv_t = const.tile([128, CH], f32, tag="bvb", name="bvb")
        dma_first(bv_t[:], bv_d[:])
        for i in range(8):
            dma_first(wq[i][:, 0:128], wq_d[i * 128 : (i + 1) * 128, 0:128])
            dma_first(wv[i][:], wv_d[i * 128 : (i + 1) * 128, :])
        # wave 2: remainders and later chunks.
        for i in range(8):
            dma_in(wk[i][:, 128:CH], wk_d[i * 128 : (i + 1) * 128, 128:CH])
            dma_in(wq[i][:, 128:CH], wq_d[i * 128 : (i + 1) * 128, 128:CH])
        for j in range(1, 4):
            dma_in(bq[j][:], bq_d[j * 128 : (j + 1) * 128, :])
            dma_in(bk[j][:], bk_d[j * 128 : (j + 1) * 128, :])
        for sc in range(1, 4):
            for i in range(8):
                dma_in(xt[i][sc][:],
                       xT_d[i * 128 : (i + 1) * 128, sc * 512 : (sc + 1) * 512])
        U_t = const.tile([128, 128], f16, tag="U", name="Ut")
        dma_in(U_t[:], U_d[:])
        R_t = const.tile([128, 896], f16, tag="R", name="Rt")
        dma_in(R_t[:], R_d[:])
        wo = [const.tile([128, D], f16, tag=f"wo{j}", name=f"wo{j}") for j in range(4)]
        for j in range(4):
            dma_in(wo[j][:], wo_d[j * 128 : (j + 1) * 128, :])
        bo_t = const.tile([128, D], f32, tag="bob", name="bob")
        dma_in(bo_t[:], bo_d[:])

        # ---- persistent activations ----
        QT = [[actp.tile([128, 512], f16, tag=f"qt{ct}_{sc}", name=f"qt{ct}_{sc}") for sc in range(4)]
              for ct in range(4)]
        KT = [[actp.tile([128, 512], f16, tag=f"kt{ct}_{sc}", name=f"kt{ct}_{sc}") for sc in range(4)]
              for ct in range(4)]
        V = [actp.tile([128, 8 * 65], f16, tag=f"v{st}", name=f"v{st}") for st in range(16)]
        OTn = [[actp.tile([128, 512], f16, tag=f"otn{hp}_{qc}", name=f"otn{hp}_{qc}") for qc in range(4)]
               for hp in range(4)]

        # ---- PE filler quanta -------------------------------------------
        # The attention kt loop leaves the PE ~40% idle (waiting on ACT exp).
        # Those slots are filled by popping emission closures from a queue:
        # the previous chunk's out-projection groups and the next chunk's
        # QKV GEMM groups.
        filler = []

        def pop_filler():
            if filler:
                filler.pop(0)()

        def drain_filler():
            while filler:
                filler.pop(0)()

        def emit_outproj_group(qc, stl, oc):
            st = 4 * qc + stl
            sl = slice(stl * 128, (stl + 1) * 128)
            ocs = slice(oc * 512, (oc + 1) * 512)
            yp = ps_p.tile([128, 512], f32, tag="p512", name="p512")
            for hpp in range(4):
                nc.tensor.matmul(yp[:], OTn[hpp][qc][:, sl],
                                 wo[hpp][:, ocs],
                                 start=(hpp == 0), stop=(hpp == 3),
                                 skip_group_check=True)
            ysb = work.tile([128, 512], f16, tag="ysb", name="ysb")
            nc.vector.scalar_tensor_tensor(ysb[:], yp[:], 1.0,
                                           bo_t[:, ocs], mult, add)
            nc.sync.dma_start(y_d[st * 128 : (st + 1) * 128, ocs], ysb[:])

        def queue_outproj(qc):
            for stl in range(4):
                for oc in range(2):
                    filler.append(
                        lambda qc=qc, stl=stl, oc=oc: emit_outproj_group(qc, stl, oc))

        def emit_kt_group(ct, sc):
            cs = slice(ct * 128, (ct + 1) * 128)
            p = ps_p.tile([128, 512], f32, tag="p512", name="p512")
            for d in range(8):
                nc.tensor.matmul(p[:], wk[d][:, cs], xt[d][sc][:],
                                 start=(d == 0), stop=(d == 7),
                                 skip_group_check=True)
            nc.vector.tensor_scalar_add(KT[ct][sc][:], p[:], bk[ct][:])

        def emit_qt_group(ct, sc):
            cs = slice(ct * 128, (ct + 1) * 128)
            p = ps_p.tile([128, 512], f32, tag="p512", name="p512")
            for d in range(8):
                nc.tensor.matmul(p[:], wq[d][:, cs], xt[d][sc][:],
                                 start=(d == 0), stop=(d == 7),
                                 skip_group_check=True)
            nc.vector.tensor_scalar_add(QT[ct][sc][:], p[:], bq[ct][:])

        def emit_v_group(stl, sc):
            st = 4 * sc + stl
            ts = slice(stl * 128, (stl + 1) * 128)
            p = ps_p.tile([128, 512], f32, tag="p512", name="p512")
            for d in range(8):
                nc.tensor.matmul(p[:], xt[d][sc][:, ts], wv[d][:, :],
                                 start=(d == 0), stop=(d == 7),
                                 skip_group_check=True)
            v3 = V[st][:].rearrange("p (h e) -> p h e", e=65)
            nc.vector.scalar_tensor_tensor(
                v3[:, :, 0:64],
                p[:].rearrange("p (h e) -> p h e", e=64),
                1.0,
                bv_t[:].rearrange("p (h e) -> p h e", e=64),
                mult, add,
            )
            nc.vector.memset(v3[:, :, 64:65], 1.0)

        def queue_qkv(sc):
            # K first (S-matmul stationary), then V, then Q.
            for ct in range(4):
                filler.append(lambda ct=ct, sc=sc: emit_kt_group(ct, sc))
            for stl in range(4):
                filler.append(lambda stl=stl, sc=sc: emit_v_group(stl, sc))
            for ct in range(4):
                filler.append(lambda ct=ct, sc=sc: emit_qt_group(ct, sc))

        # ---- chunk 0 QKV: only what hp0's attention needs runs up front
        # (KT/QT ct=0 + all V); the other head-pairs' K/Q projections are
        # queued as filler consumed during earlier head-pairs' kt loops.
        emit_kt_group(0, 0)
        emit_qt_group(0, 0)
        for stl in range(4):
            emit_v_group(stl, 0)

        for sc in range(4):
            # Queue this chunk's deferred PE work: previous chunk's
            # out-projection, then the NEXT chunk's QKV projections.
            if sc == 0:
                for ct in range(1, 4):
                    filler.append(lambda ct=ct: emit_kt_group(ct, 0))
                    filler.append(lambda ct=ct: emit_qt_group(ct, 0))
            if sc > 0:
                queue_outproj(sc - 1)
            if sc < 3:
                queue_qkv(sc + 1)

            # attention for query chunk qc = sc, head pairs interleaved so the
            # even head's K=64 matmuls (rows 0-63) and the odd head's (rows
            # 64-127) run concurrently in the PE array.
            qc = sc
            nkt = 4 * (qc + 1)
            for hp in range(4):
                h0, h1 = 2 * hp, 2 * hp + 1
                av0 = ps_p.tile([65, 512], f32, tag="p512", name="av0")
                av1 = ps_p.tile([65, 512], f32, tag="p512", name="av1")

                # software pipeline: S(kt) -> exp(kt) on ACT while PE runs
                # S(kt+1); AV(kt) issues after S(kt+1) so PE never waits exp.
                pend = []  # (kt, pt tile, delta) awaiting AV

                def emit_av(kt, pt, delta, first, last):
                    nc.tensor.matmul(
                        av0[:, delta:512], V[kt][:, h0 * 65 : h0 * 65 + 65],
                        pt[:, delta:512],
                        start=first, stop=last, skip_group_check=True)
                    nc.tensor.matmul(
                        av1[:, delta:512], V[kt][:, h1 * 65 : h1 * 65 + 65],
                        pt[:, 512 + delta : 1024],
                        start=first, stop=last, skip_group_check=True)

                for kt in range(nkt):
                    # merged S^T tile: cols 0-511 head h0, 512-1023 head h1.
                    # Diagonal-straddling blocks: cols < delta are fully
                    # masked -> not computed at all; the 128-wide strip
                    # [delta, delta+128) gets the -BIG ramp added.
                    diag = kt >= 4 * qc
                    delta = 128 * kt - 512 * qc if diag else 0
                    sp = ps_s.tile([128, 1024], f32, tag="s2", name="sp")
                    kcol = slice((kt % 4) * 128, (kt % 4) * 128 + 128)
                    nc.tensor.matmul(
                        sp[:, delta:512], KT[hp][kt // 4][0:64, kcol],
                        QT[hp][qc][0:64, delta:512],
                        start=True, stop=not diag, skip_group_check=True)
                    nc.tensor.matmul(
                        sp[:, 512 + delta : 1024], KT[hp][kt // 4][64:128, kcol],
                        QT[hp][qc][64:128, delta:512],
                        start=True, stop=not diag, skip_group_check=True)
                    if diag:
                        rsl = slice(384, 512)
                        nc.tensor.matmul(sp[:, delta : delta + 128], U_t[:],
                                         R_t[:, rsl],
                                         start=False, stop=True,
                                         skip_group_check=True)
                        nc.tensor.matmul(sp[:, 512 + delta : 512 + delta + 128],
                                         U_t[:], R_t[:, rsl],
                                         start=False, stop=True,
                                         skip_group_check=True)
                    pt = ptp.tile([128, 1024], f16, tag="pt", name="pt")
                    sp3 = sp[:].rearrange("p (h q) -> p h q", h=2)
                    pt3 = pt[:].rearrange("p (h q) -> p h q", h=2)
                    nc.scalar.activation(pt3[:, :, delta:512],
                                         sp3[:, :, delta:512], Exp, scale=SCALE)
                    pend.append((kt, pt, delta))
                    pop_filler()
                    # AVs emitted in 2-kt batches: fewer S<->AV stationary
                    # switches means fewer un-hidden LDWEIGHTS on the PE.
                    if kt % 2 == 1 and len(pend) > 2:
                        for _ in range(2):
                            k0, p0, d0 = pend.pop(0)
                            emit_av(k0, p0, d0, k0 == 0, False)
                while pend:
                    k0, p0, d0 = pend.pop(0)
                    emit_av(k0, p0, d0, k0 == 0, not pend)

                # softmax normalization: evacuate av PSUM to SBUF right away
                # (frees the PSUM pool), then normalize from SBUF off the
                # critical path. Row 64 of av = softmax denominator.
                avsb = normp.tile([65, 1024], f32, tag="avsb", name="avsb")
                nc.vector.tensor_copy(avsb[:, 0:512], av0[:])
                nc.vector.tensor_copy(avsb[:, 512:1024], av1[:])
                r = work.tile([1, 1024], f32, tag="r", name="r")
                if qc < 3 or hp == 3:
                    # 1/d = exp(-ln(d)) on ACT (both fns share one table
                    # set). ACT has slack in the PE-bound chunks and after
                    # the final chunk's last exp; the [1,N] reciprocal on
                    # DVE would throttle the chunk cadence instead.
                    lnt = work.tile([1, 1024], f32, tag="lnt", name="lnt")
                    nc.scalar.activation(lnt[:], avsb[64:65, :], Ln)
                    nc.scalar.activation(r[:], lnt[:], Exp, scale=-1.0)
                else:
                    nc.vector.reciprocal(r[:, 0:512], avsb[64:65, 0:512])
                    nc.vector.reciprocal(r[:, 512:1024], avsb[64:65, 512:1024])
                rb0 = work.tile([64, 512], f32, tag="rb", name="rb0")
                nc.gpsimd.partition_broadcast(rb0[:], r[0:1, 0:512], channels=64)
                nc.vector.tensor_mul(OTn[hp][qc][0:64, :], avsb[0:64, 0:512],
                                     rb0[:])
                rb1 = work.tile([64, 512], f32, tag="rb", name="rb1")
                nc.gpsimd.partition_broadcast(rb1[:], r[0:1, 512:1024], channels=64)
                nc.vector.tensor_mul(OTn[hp][qc][64:128, :], avsb[0:64, 512:1024],
                                     rb1[:])
            drain_filler()
        queue_outproj(3)
        drain_filler()

    nc.compile()
    bacc.get_activation_tables = orig_tables
    return nc


def _host_inputs(x, w_qkv, b_qkv, w_o, b_o):
    """Per-core input dicts implementing the sharding + layout prep."""
    U = np.zeros((128, 128), np.float16)
    for c in range(128):
        U[c, c:] = 1.0
    R = np.zeros((128, 896), np.float16)
    for c in range(128):
        R[c, : c + 384] = -BIG

    in_maps = []
    for c in range(N_CORES):
        b = c // 2
        hs = (c % 2) * HPC
        cols = slice(hs * DH, (hs + HPC) * DH)
        in_maps.append({
            "xT": np.ascontiguousarray(x[b].T).astype(np.float16),
            "wq": w_qkv[:, cols].astype(np.float16),
            "wk": w_qkv[:, D:][:, cols].astype(np.float16),
            "wv": w_qkv[:, 2 * D:][:, cols].astype(np.float16),
            "wo": w_o[hs * DH : (hs + HPC) * DH, :].astype(np.float16),
            "bq": b_qkv[cols].reshape(CH, 1).astype(np.float32),
            "bk": b_qkv[D:][cols].reshape(CH, 1).astype(np.float32),
            "bvb": np.tile(b_qkv[2 * D:][cols].astype(np.float32), (128, 1)),
            "bob": np.tile(b_o.astype(np.float32), (128, 1)),
            "U": U,
            "R": R,
        })
    return in_maps


def kernel(x, w_qkv, b_qkv, w_o, b_o):
    global _cached
    from concourse.bass_utils import run_bass_kernel_spmd

    x = np.asarray(x)
    w_qkv = np.asarray(w_qkv)
    b_qkv = np.asarray(b_qkv)
    w_o = np.asarray(w_o)
    b_o = np.asarray(b_o)

    if _cached is None:
        _cached = _build_program()
    nc = _cached

    in_maps = _host_inputs(x, w_qkv, b_qkv, w_o, b_o)
    res = run_bass_kernel_spmd(nc, in_maps, list(range(N_CORES)))

    out = np.empty((B, N, D), np.float32)
    for b in range(B):
        out[b] = (res.results[2 * b]["y"].astype(np.float32)
                  + res.results[2 * b + 1]["y"].astype(np.float32))
    return out


# revision 17
# speedup vs baseline: 1.2484x; 1.0654x over previous
"""Causal self-attention on 8 Trainium2 NeuronCores.

Sharding: core c handles batch b = c//2 and heads [(c%2)*8, (c%2)*8+8).
Each core computes the full QKV projection for its head slice, causal
flash-style attention, and the row-parallel w_o partial product. The two
partials per batch are summed on the host (no device collectives).

All PE matmuls run in fp16 (1 cycle/row) with fp32 PSUM accumulation.
Feature-major layouts throughout:
  x^T [D, N]        (host pre-transposed)
  Q^T, K^T [ch, N]  (from GEMM with W stationary, x^T moving)
  V [N, ch] + ones  (from GEMM with x^T stationary, W moving)
  S^T [k, q] = K^T_tile.T @ Q^T  -> exp -> P^T [k, q]
  O^T [ch, q] = (V|1).T @ P^T    (row 64 = softmax denominator)
  y = O^T_norm.T @ W_o           (accumulated over ch tiles)

Causal masking: diagonal-straddling S^T blocks only compute columns
>= delta (cols below are fully masked); the 128-wide partial strip gets
-BIG * max(k - u, 0) added via an extra accumulating matmul so exp()
underflows to exact zeros.

The attention kt loop is ACT(exp)-bound; PE idle inside it is filled by
interleaving the previous chunk's out-projection and the next chunk's
QKV matmuls as filler quanta.
"""

import numpy as np

B, N, D, H = 4, 2048, 1024, 16
DH = 64
N_CORES = 8
HPC = 8            # heads per core
CH = HPC * DH      # 512 channels per core
SCALE = 1.0 / 8.0  # 1/sqrt(DH)
BIG = 280.0        # SCALE*BIG = 35 >> max |S/8|, exp underflows to 0

_cached = None


def _build_program():
    from contextlib import ExitStack

    import concourse.tile as tile
    from concourse import bacc, mybir

    f16 = mybir.dt.float16
    f32 = mybir.dt.float32
    Exp = mybir.ActivationFunctionType.Exp
    Ln = mybir.ActivationFunctionType.Ln
    mult = mybir.AluOpType.mult
    add = mybir.AluOpType.add

    # The kernel uses both Exp (softmax) and Ln (reciprocal-via-exp(-ln)).
    # The table-load placement pass assigns each activation the first set
    # containing its function, which thrashes ~27 ACT_TABLE_LOADs between
    # `exp_and_others` and `natural_log`. Restrict Exp/Ln to the combined
    # `natural_log_exp_and_others` set (names/order unchanged, so the
    # act_func_set_id indexing stays valid) -> exactly one load.
    orig_tables = bacc.get_activation_tables

    def _patched_tables(arch):
        t = dict(orig_tables(arch))
        for name, fns in t.items():
            if name != "natural_log_exp_and_others":
                t[name] = {
                    f for f in fns
                    if f not in (mybir.ActivationFunctionType.Exp,
                                 mybir.ActivationFunctionType.Ln)
                }
        return t

    bacc.get_activation_tables = _patched_tables

    nc = bacc.Bacc(
        "TRN2", target_bir_lowering=False, debug=False, num_devices=N_CORES
    )

    xT_d = nc.dram_tensor("xT", [D, N], f16, kind="ExternalInput").ap()
    wq_d = nc.dram_tensor("wq", [D, CH], f16, kind="ExternalInput").ap()
    wk_d = nc.dram_tensor("wk", [D, CH], f16, kind="ExternalInput").ap()
    wv_d = nc.dram_tensor("wv", [D, CH], f16, kind="ExternalInput").ap()
    wo_d = nc.dram_tensor("wo", [CH, D], f16, kind="ExternalInput").ap()
    bq_d = nc.dram_tensor("bq", [CH, 1], f32, kind="ExternalInput").ap()
    bk_d = nc.dram_tensor("bk", [CH, 1], f32, kind="ExternalInput").ap()
    bv_d = nc.dram_tensor("bvb", [128, CH], f32, kind="ExternalInput").ap()
    bo_d = nc.dram_tensor("bob", [128, D], f32, kind="ExternalInput").ap()
    U_d = nc.dram_tensor("U", [128, 128], f16, kind="ExternalInput").ap()
    R_d = nc.dram_tensor("R", [128, 896], f16, kind="ExternalInput").ap()
    y_d = nc.dram_tensor("y", [N, D], f16, kind="ExternalOutput").ap()

    with tile.TileContext(nc) as tc, ExitStack() as ctx:
        const = ctx.enter_context(tc.tile_pool(name="const", bufs=1))
        actp = ctx.enter_context(tc.tile_pool(name="actp", bufs=1))
        work = ctx.enter_context(tc.tile_pool(name="work", bufs=3))
        ptp = ctx.enter_context(tc.tile_pool(name="ptp", bufs=5))
        normp = ctx.enter_context(tc.tile_pool(name="normp", bufs=2))
        ps_s = ctx.enter_context(tc.tile_pool(name="ps_s", bufs=2, space="PSUM"))
        ps_p = ctx.enter_context(tc.tile_pool(name="ps_p", bufs=4, space="PSUM"))

        # ---- constants / weights into SBUF ----
        # K-weights + first seq-chunk of x first so the K^T GEMM starts ASAP.
        wq = [const.tile([128, CH], f16, tag=f"wq{i}", name=f"wq{i}") for i in range(8)]
        wk = [const.tile([128, CH], f16, tag=f"wk{i}", name=f"wk{i}") for i in range(8)]
        wv = [const.tile([128, CH], f16, tag=f"wv{i}", name=f"wv{i}") for i in range(8)]
        xt = [[const.tile([128, 512], f16, tag=f"xt{i}_{sc}", name=f"xt{i}_{sc}")
               for sc in range(4)] for i in range(8)]
        # Round-robin input DMAs across engine queues so the 2D row-descriptor
        # processing runs in parallel; first-needed first. The first wave
        # (wk + first x chunk) additionally uses the vector/scalar queues,
        # which are compute-idle until the first GEMM finishes.
        first_engs = [nc.sync, nc.gpsimd, nc.scalar]
        engs = [nc.sync, nc.gpsimd]
        _ei = [0]

        def dma_first(dst, src):
            first_engs[_ei[0] % len(first_engs)].dma_start(dst, src)
            _ei[0] += 1

        def dma_in(dst, src):
            engs[_ei[0] % len(engs)].dma_start(dst, src)
            _ei[0] += 1

        # wave 1: just the ct=0 column slices of wk/wq + x chunk 0 + wv,
        # so the first KT/QT/V GEMMs can start after ~2.5MB instead of ~5MB.
        for i in range(8):
            dma_first(wk[i][:, 0:128], wk_d[i * 128 : (i + 1) * 128, 0:128])
            dma_first(xt[i][0][:], xT_d[i * 128 : (i + 1) * 128, 0:512])
        bq = [const.tile([128, 1], f32, tag=f"bq{j}", name=f"bq{j}") for j in range(4)]
        bk = [const.tile([128, 1], f32, tag=f"bk{j}", name=f"bk{j}") for j in range(4)]
        dma_first(bk[0][:], bk_d[0:128, :])
        dma_first(bq[0][:], bq_d[0:128, :])
        bv_t = const.tile([128, CH], f32, tag="bvb", name="bvb")
        dma_first(bv_t[:], bv_d[:])
        for i in range(8):
            dma_first(wq[i][:, 0:128], wq_d[i * 128 : (i + 1) * 128, 0:128])
            dma_first(wv[i][:], wv_d[i * 128 : (i + 1) * 128, :])
        # wave 2: remainders and later chunks.
        for i in range(8):
            dma_in(wk[i][:, 128:CH], wk_d[i * 128 : (i + 1) * 128, 128:CH])
            dma_in(wq[i][:, 128:CH], wq_d[i * 128 : (i + 1) * 128, 128:CH])
        for j in range(1, 4):
            dma_in(bq[j][:], bq_d[j * 128 : (j + 1) * 128, :])
            dma_in(bk[j][:], bk_d[j * 128 : (j + 1) * 128, :])
        for sc in range(1, 4):
            for i in range(8):
                dma_in(xt[i][sc][:],
                       xT_d[i * 128 : (i + 1) * 128, sc * 512 : (sc + 1) * 512])
        U_t = const.tile([128, 128], f16, tag="U", name="Ut")
        dma_in(U_t[:], U_d[:])
        R_t = const.tile([128, 896], f16, tag="R", name="Rt")
        dma_in(R_t[:], R_d[:])
        wo = [const.tile([128, D], f16, tag=f"wo{j}", name=f"wo{j}") for j in range(4)]
        for j in range(4):
            dma_in(wo[j][:], wo_d[j * 128 : (j + 1) * 128, :])
        bo_t = const.tile([128, D], f32, tag="bob", name="bob")
        dma_in(bo_t[:], bo_d[:])

        # ---- persistent activations ----
        QT = [[actp.tile([128, 512], f16, tag=f"qt{ct}_{sc}", name=f"qt{ct}_{sc}") for sc in range(4)]
              for ct in range(4)]
        KT = [[actp.tile([128, 512], f16, tag=f"kt{ct}_{sc}", name=f"kt{ct}_{sc}") for sc in range(4)]
              for ct in range(4)]
        V = [actp.tile([128, 8 * 65], f16, tag=f"v{st}", name=f"v{st}") for st in range(16)]
        OTn = [[actp.tile([128, 512], f16, tag=f"otn{hp}_{qc}", name=f"otn{hp}_{qc}") for qc in range(4)]
               for hp in range(4)]

        # ---- PE filler quanta -------------------------------------------
        # The attention kt loop leaves the PE ~40% idle (waiting on ACT exp).
        # Those slots are filled by popping emission closures from a queue:
        # the previous chunk's out-projection groups and the next chunk's
        # QKV GEMM groups.
        filler = []

        def pop_filler():
            if filler:
                filler.pop(0)()

        def drain_filler():
            while filler:
                filler.pop(0)()

        def emit_outproj_group(qc, stl, oc):
            st = 4 * qc + stl
            sl = slice(stl * 128, (stl + 1) * 128)
            ocs = slice(oc * 512, (oc + 1) * 512)
            yp = ps_p.tile([128, 512], f32, tag="p512", name="p512")
            for hpp in range(4):
                nc.tensor.matmul(yp[:], OTn[hpp][qc][:, sl],
                                 wo[hpp][:, ocs],
                                 start=(hpp == 0), stop=(hpp == 3),
                                 skip_group_check=True)
            ysb = work.tile([128, 512], f16, tag="ysb", name="ysb")
            nc.vector.scalar_tensor_tensor(ysb[:], yp[:], 1.0,
                                           bo_t[:, ocs], mult, add)
            nc.sync.dma_start(y_d[st * 128 : (st + 1) * 128, ocs], ysb[:])

        def queue_outproj(qc):
            for stl in range(4):
                for oc in range(2):
                    filler.append(
                        lambda qc=qc, stl=stl, oc=oc: emit_outproj_group(qc, stl, oc))

        def emit_kt_group(ct, sc):
            cs = slice(ct * 128, (ct + 1) * 128)
            p = ps_p.tile([128, 512], f32, tag="p512", name="p512")
            for d in range(8):
                nc.tensor.matmul(p[:], wk[d][:, cs], xt[d][sc][:],
                                 start=(d == 0), stop=(d == 7),
                                 skip_group_check=True)
            nc.vector.tensor_scalar_add(KT[ct][sc][:], p[:], bk[ct][:])

        def emit_qt_group(ct, sc):
            cs = slice(ct * 128, (ct + 1) * 128)
            p = ps_p.tile([128, 512], f32, tag="p512", name="p512")
            for d in range(8):
                nc.tensor.matmul(p[:], wq[d][:, cs], xt[d][sc][:],
                                 start=(d == 0), stop=(d == 7),
                                 skip_group_check=True)
            nc.vector.tensor_scalar_add(QT[ct][sc][:], p[:], bq[ct][:])

        def emit_v_group(stl, sc):
            st = 4 * sc + stl
            ts = slice(stl * 128, (stl + 1) * 128)
            p = ps_p.tile([128, 512], f32, tag="p512", name="p512")
            for d in range(8):
                nc.tensor.matmul(p[:], xt[d][sc][:, ts], wv[d][:, :],
                                 start=(d == 0), stop=(d == 7),
                                 skip_group_check=True)
            v3 = V[st][:].rearrange("p (h e) -> p h e", e=65)
            nc.vector.scalar_tensor_tensor(
                v3[:, :, 0:64],
                p[:].rearrange("p (h e) -> p h e", e=64),
                1.0,
                bv_t[:].rearrange("p (h e) -> p h e", e=64),
                mult, add,
            )
            nc.vector.memset(v3[:, :, 64:65], 1.0)

        def queue_qkv(sc):
            # K first (S-matmul stationary), then V, then Q.
            for ct in range(4):
                filler.append(lambda ct=ct, sc=sc: emit_kt_group(ct, sc))
            for stl in range(4):
                filler.append(lambda stl=stl, sc=sc: emit_v_group(stl, sc))
            for ct in range(4):
                filler.append(lambda ct=ct, sc=sc: emit_qt_group(ct, sc))

        # ---- attention software pipeline state (spans hp/chunk boundaries) --
        pend = []    # entries awaiting their AV matmuls
        flip = [0]

        def emit_norm(e):
            # softmax normalization: evacuate av PSUM to SBUF right away
            # (frees the PSUM pool), then normalize from SBUF off the
            # critical path. Row 64 of av = softmax denominator.
            hp, qc = e["hp"], e["qc"]
            avsb = normp.tile([65, 1024], f32, tag="avsb", name="avsb")
            nc.vector.tensor_copy(avsb[:, 0:512], e["av0"][:])
            nc.vector.tensor_copy(avsb[:, 512:1024], e["av1"][:])
            r = work.tile([1, 1024], f32, tag="r", name="r")
            if qc < 3 or hp == 3:
                # 1/d = exp(-ln(d)) on ACT (both fns share one table set).
                # ACT has slack in the PE-bound chunks and after the final
                # chunk's last exp; the [1,N] reciprocal on DVE would
                # throttle the chunk cadence instead.
                lnt = work.tile([1, 1024], f32, tag="lnt", name="lnt")
                nc.scalar.activation(lnt[:], avsb[64:65, :], Ln)
                nc.scalar.activation(r[:], lnt[:], Exp, scale=-1.0)
            else:
                nc.vector.reciprocal(r[:, 0:512], avsb[64:65, 0:512])
                nc.vector.reciprocal(r[:, 512:1024], avsb[64:65, 512:1024])
            rb0 = work.tile([64, 512], f32, tag="rb", name="rb0")
            nc.gpsimd.partition_broadcast(rb0[:], r[0:1, 0:512], channels=64)
            nc.vector.tensor_mul(OTn[hp][qc][0:64, :], avsb[0:64, 0:512],
                                 rb0[:])
            rb1 = work.tile([64, 512], f32, tag="rb", name="rb1")
            nc.gpsimd.partition_broadcast(rb1[:], r[0:1, 512:1024], channels=64)
            nc.vector.tensor_mul(OTn[hp][qc][64:128, :], avsb[0:64, 512:1024],
                                 rb1[:])

        def flush_av():
            e = pend.pop(0)
            d = e["delta"]
            nc.tensor.matmul(
                e["av0"][:, d:512], V[e["kt"]][:, e["h0"] * 65 : e["h0"] * 65 + 65],
                e["pt"][:, d:512],
                start=e["first"], stop=e["last"], skip_group_check=True)
            nc.tensor.matmul(
                e["av1"][:, d:512], V[e["kt"]][:, e["h1"] * 65 : e["h1"] * 65 + 65],
                e["pt"][:, 512 + d : 1024],
                start=e["first"], stop=e["last"], skip_group_check=True)
            if e["last"]:
                emit_norm(e)

        # ---- chunk 0 QKV: only what hp0's attention needs runs up front
        # (KT/QT ct=0 + all V); the other head-pairs' K/Q projections are
        # queued as filler consumed during earlier head-pairs' kt loops.
        emit_kt_group(0, 0)
        emit_qt_group(0, 0)
        for stl in range(4):
            emit_v_group(stl, 0)

        for sc in range(4):
            # Queue this chunk's deferred PE work: previous chunk's
            # out-projection, then the NEXT chunk's QKV projections.
            if sc == 0:
                for ct in range(1, 4):
                    filler.append(lambda ct=ct: emit_kt_group(ct, 0))
                    filler.append(lambda ct=ct: emit_qt_group(ct, 0))
            # QKV(sc+1) first: the out-projection quanta need ALL of chunk
            # sc-1's OTn tiles, whose last normalization lands a few us into
            # this chunk -- popping them later avoids a PE stall. In the last
            # chunk (no QKV left, PE otherwise starved) outproj goes first.
            if sc < 3:
                queue_qkv(sc + 1)
            if sc > 0:
                queue_outproj(sc - 1)

            # attention for query chunk qc = sc, head pairs interleaved so the
            # even head's K=64 matmuls (rows 0-63) and the odd head's (rows
            # 64-127) run concurrently in the PE array. The S->exp->AV
            # software pipeline (pend) runs FLAT across head-pair and chunk
            # boundaries so the ACT exp stream never drains at a boundary.
            qc = sc
            nkt = 4 * (qc + 1)
            for hp in range(4):
                h0, h1 = 2 * hp, 2 * hp + 1
                av0 = ps_p.tile([65, 512], f32, tag="p512", name="av0")
                av1 = ps_p.tile([65, 512], f32, tag="p512", name="av1")

                for kt in range(nkt):
                    # merged S^T tile: cols 0-511 head h0, 512-1023 head h1.
                    # Diagonal-straddling blocks: cols < delta are fully
                    # masked -> not computed at all; the 128-wide strip
                    # [delta, delta+128) gets the -BIG ramp added.
                    diag = kt >= 4 * qc
                    delta = 128 * kt - 512 * qc if diag else 0
                    sp = ps_s.tile([128, 1024], f32, tag="s2", name="sp")
                    kcol = slice((kt % 4) * 128, (kt % 4) * 128 + 128)
                    nc.tensor.matmul(
                        sp[:, delta:512], KT[hp][kt // 4][0:64, kcol],
                        QT[hp][qc][0:64, delta:512],
                        start=True, stop=not diag, skip_group_check=True)
                    nc.tensor.matmul(
                        sp[:, 512 + delta : 1024], KT[hp][kt // 4][64:128, kcol],
                        QT[hp][qc][64:128, delta:512],
                        start=True, stop=not diag, skip_group_check=True)
                    if diag:
                        rsl = slice(384, 512)
                        nc.tensor.matmul(sp[:, delta : delta + 128], U_t[:],
                                         R_t[:, rsl],
                                         start=False, stop=True,
                                         skip_group_check=True)
                        nc.tensor.matmul(sp[:, 512 + delta : 512 + delta + 128],
                                         U_t[:], R_t[:, rsl],
                                         start=False, stop=True,
                                         skip_group_check=True)
                    pt = ptp.tile([128, 1024], f16, tag="pt", name="pt")
                    sp3 = sp[:].rearrange("p (h q) -> p h q", h=2)
                    pt3 = pt[:].rearrange("p (h q) -> p h q", h=2)
                    nc.scalar.activation(pt3[:, :, delta:512],
                                         sp3[:, :, delta:512], Exp, scale=SCALE)
                    pend.append(dict(kt=kt, pt=pt, delta=delta, av0=av0,
                                     av1=av1, h0=h0, h1=h1, hp=hp, qc=qc,
                                     first=(kt == 0), last=(kt == nkt - 1)))
                    pop_filler()
                    # AVs trail the exp stream by >=2 entries and are emitted
                    # in batches of 2: fewer S<->AV stationary switches means
                    # fewer un-hidden LDWEIGHTS on the PE.
                    flip[0] ^= 1
                    if flip[0] == 0:
                        while len(pend) > 2:
                            flush_av()
            drain_filler()
        while pend:
            flush_av()
        queue_outproj(3)
        drain_filler()

    nc.compile()
    bacc.get_activation_tables = orig_tables
    return nc


def _host_inputs(x, w_qkv, b_qkv, w_o, b_o):
    """Per-core input dicts implementing the sharding + layout prep."""
    U = np.zeros((128, 128), np.float16)
    for c in range(128):
        U[c, c:] = 1.0
    R = np.zeros((128, 896), np.float16)
    for c in range(128):
        R[c, : c + 384] = -BIG

    in_maps = []
    for c in range(N_CORES):
        b = c // 2
        hs = (c % 2) * HPC
        cols = slice(hs * DH, (hs + HPC) * DH)
        in_maps.append({
            "xT": np.ascontiguousarray(x[b].T).astype(np.float16),
            "wq": w_qkv[:, cols].astype(np.float16),
            "wk": w_qkv[:, D:][:, cols].astype(np.float16),
            "wv": w_qkv[:, 2 * D:][:, cols].astype(np.float16),
            "wo": w_o[hs * DH : (hs + HPC) * DH, :].astype(np.float16),
            "bq": b_qkv[cols].reshape(CH, 1).astype(np.float32),
            "bk": b_qkv[D:][cols].reshape(CH, 1).astype(np.float32),
            "bvb": np.tile(b_qkv[2 * D:][cols].astype(np.float32), (128, 1)),
            "bob": np.tile(b_o.astype(np.float32), (128, 1)),
            "U": U,
            "R": R,
        })
    return in_maps


def kernel(x, w_qkv, b_qkv, w_o, b_o):
    global _cached
    from concourse.bass_utils import run_bass_kernel_spmd

    x = np.asarray(x)
    w_qkv = np.asarray(w_qkv)
    b_qkv = np.asarray(b_qkv)
    w_o = np.asarray(w_o)
    b_o = np.asarray(b_o)

    if _cached is None:
        _cached = _build_program()
    nc = _cached

    in_maps = _host_inputs(x, w_qkv, b_qkv, w_o, b_o)
    res = run_bass_kernel_spmd(nc, in_maps, list(range(N_CORES)))

    out = np.empty((B, N, D), np.float32)
    for b in range(B):
        out[b] = (res.results[2 * b]["y"].astype(np.float32)
                  + res.results[2 * b + 1]["y"].astype(np.float32))
    return out


# revision 21
# speedup vs baseline: 1.2682x; 1.0158x over previous
"""Causal self-attention on 8 Trainium2 NeuronCores.

Sharding: core c handles batch b = c//2 and heads [(c%2)*8, (c%2)*8+8).
Each core computes the full QKV projection for its head slice, causal
flash-style attention, and the row-parallel w_o partial product. The two
partials per batch are summed on the host (no device collectives).

All PE matmuls run in fp16 (1 cycle/row) with fp32 PSUM accumulation.
Feature-major layouts throughout:
  x^T [D, N]        (host pre-transposed)
  Q^T, K^T [ch, N]  (from GEMM with W stationary, x^T moving)
  V [N, ch] + ones  (from GEMM with x^T stationary, W moving)
  S^T [k, q] = K^T_tile.T @ Q^T  -> exp -> P^T [k, q]
  O^T [ch, q] = (V|1).T @ P^T    (row 64 = softmax denominator)
  y = O^T_norm.T @ W_o           (accumulated over ch tiles)

Causal masking: diagonal-straddling S^T blocks only compute columns
>= delta (cols below are fully masked); the 128-wide partial strip gets
-BIG * max(k - u, 0) added via an extra accumulating matmul so exp()
underflows to exact zeros.

The attention kt loop is ACT(exp)-bound; PE idle inside it is filled by
interleaving the previous chunk's out-projection and the next chunk's
QKV matmuls as filler quanta.
"""

import numpy as np

B, N, D, H = 4, 2048, 1024, 16
DH = 64
N_CORES = 8
HPC = 8            # heads per core
CH = HPC * DH      # 512 channels per core
SCALE = 1.0 / 8.0  # 1/sqrt(DH)
BIG = 280.0        # SCALE*BIG = 35 >> max |S/8|, exp underflows to 0

_cached = None


def _build_program():
    from contextlib import ExitStack

    import concourse.tile as tile
    from concourse import bacc, mybir

    f16 = mybir.dt.float16
    f32 = mybir.dt.float32
    Exp = mybir.ActivationFunctionType.Exp
    Ln = mybir.ActivationFunctionType.Ln
    mult = mybir.AluOpType.mult
    add = mybir.AluOpType.add

    # The kernel uses both Exp (softmax) and Ln (reciprocal-via-exp(-ln)).
    # The table-load placement pass assigns each activation the first set
    # containing its function, which thrashes ~27 ACT_TABLE_LOADs between
    # `exp_and_others` and `natural_log`. Restrict Exp/Ln to the combined
    # `natural_log_exp_and_others` set (names/order unchanged, so the
    # act_func_set_id indexing stays valid) -> exactly one load.
    orig_tables = bacc.get_activation_tables

    def _patched_tables(arch):
        t = dict(orig_tables(arch))
        for name, fns in t.items():
            if name != "natural_log_exp_and_others":
                t[name] = {
                    f for f in fns
                    if f not in (mybir.ActivationFunctionType.Exp,
                                 mybir.ActivationFunctionType.Ln)
                }
        return t

    bacc.get_activation_tables = _patched_tables

    nc = bacc.Bacc(
        "TRN2", target_bir_lowering=False, debug=False, num_devices=N_CORES
    )

    xT_d = nc.dram_tensor("xT", [D, N], f16, kind="ExternalInput").ap()
    wq_d = nc.dram_tensor("wq", [D, CH], f16, kind="ExternalInput").ap()
    wk_d = nc.dram_tensor("wk", [D, CH], f16, kind="ExternalInput").ap()
    wv_d = nc.dram_tensor("wv", [D, CH], f16, kind="ExternalInput").ap()
    wo_d = nc.dram_tensor("wo", [CH, D], f16, kind="ExternalInput").ap()
    bq_d = nc.dram_tensor("bq", [CH, 1], f32, kind="ExternalInput").ap()
    bk_d = nc.dram_tensor("bk", [CH, 1], f32, kind="ExternalInput").ap()
    bv_d = nc.dram_tensor("bvb", [128, CH], f32, kind="ExternalInput").ap()
    bo_d = nc.dram_tensor("bob", [128, D], f32, kind="ExternalInput").ap()
    U_d = nc.dram_tensor("U", [128, 128], f16, kind="ExternalInput").ap()
    R_d = nc.dram_tensor("R", [128, 896], f16, kind="ExternalInput").ap()
    y_d = nc.dram_tensor("y", [N, D], f16, kind="ExternalOutput").ap()

    with tile.TileContext(nc) as tc, ExitStack() as ctx:
        const = ctx.enter_context(tc.tile_pool(name="const", bufs=1))
        actp = ctx.enter_context(tc.tile_pool(name="actp", bufs=1))
        work = ctx.enter_context(tc.tile_pool(name="work", bufs=3))
        ptp = ctx.enter_context(tc.tile_pool(name="ptp", bufs=5))
        normp = ctx.enter_context(tc.tile_pool(name="normp", bufs=2))
        ps_s = ctx.enter_context(tc.tile_pool(name="ps_s", bufs=2, space="PSUM"))
        ps_p = ctx.enter_context(tc.tile_pool(name="ps_p", bufs=4, space="PSUM"))

        # ---- constants / weights into SBUF ----
        # K-weights + first seq-chunk of x first so the K^T GEMM starts ASAP.
        wq = [const.tile([128, CH], f16, tag=f"wq{i}", name=f"wq{i}") for i in range(8)]
        wk = [const.tile([128, CH], f16, tag=f"wk{i}", name=f"wk{i}") for i in range(8)]
        wv = [const.tile([128, CH], f16, tag=f"wv{i}", name=f"wv{i}") for i in range(8)]
        xt = [[const.tile([128, 512], f16, tag=f"xt{i}_{sc}", name=f"xt{i}_{sc}")
               for sc in range(4)] for i in range(8)]
        # Round-robin input DMAs across engine queues so the 2D row-descriptor
        # processing runs in parallel; first-needed first. The first wave
        # (wk + first x chunk) additionally uses the vector/scalar queues,
        # which are compute-idle until the first GEMM finishes.
        first_engs = [nc.sync, nc.gpsimd, nc.scalar]
        engs = [nc.sync, nc.gpsimd]
        _ei = [0]

        def dma_first(dst, src):
            first_engs[_ei[0] % len(first_engs)].dma_start(dst, src)
            _ei[0] += 1

        def dma_in(dst, src):
            engs[_ei[0] % len(engs)].dma_start(dst, src)
            _ei[0] += 1

        # wave 1: just the ct=0 column slices of wk/wq + x chunk 0 + wv,
        # so the first KT/QT/V GEMMs can start after ~2.5MB instead of ~5MB.
        for i in range(8):
            dma_first(wk[i][:, 0:128], wk_d[i * 128 : (i + 1) * 128, 0:128])
            dma_first(xt[i][0][:], xT_d[i * 128 : (i + 1) * 128, 0:512])
        bq = [const.tile([128, 1], f32, tag=f"bq{j}", name=f"bq{j}") for j in range(4)]
        bk = [const.tile([128, 1], f32, tag=f"bk{j}", name=f"bk{j}") for j in range(4)]
        dma_first(bk[0][:], bk_d[0:128, :])
        dma_first(bq[0][:], bq_d[0:128, :])
        bv_t = const.tile([128, CH], f32, tag="bvb", name="bvb")
        dma_first(bv_t[:], bv_d[:])
        for i in range(8):
            dma_first(wq[i][:, 0:128], wq_d[i * 128 : (i + 1) * 128, 0:128])
            dma_first(wv[i][:], wv_d[i * 128 : (i + 1) * 128, :])
        # U/R are tiny (256KB) and attention(0)'s diagonal masking needs
        # them right after the first QKV groups -> load before wave 2.
        U_t = const.tile([128, 128], f16, tag="U", name="Ut")
        dma_first(U_t[:], U_d[:])
        R_t = const.tile([128, 896], f16, tag="R", name="Rt")
        dma_first(R_t[:], R_d[:])
        # wave 2: remainders and later chunks.
        for i in range(8):
            dma_in(wk[i][:, 128:CH], wk_d[i * 128 : (i + 1) * 128, 128:CH])
            dma_in(wq[i][:, 128:CH], wq_d[i * 128 : (i + 1) * 128, 128:CH])
        for j in range(1, 4):
            dma_in(bq[j][:], bq_d[j * 128 : (j + 1) * 128, :])
            dma_in(bk[j][:], bk_d[j * 128 : (j + 1) * 128, :])
        for sc in range(1, 4):
            for i in range(8):
                dma_in(xt[i][sc][:],
                       xT_d[i * 128 : (i + 1) * 128, sc * 512 : (sc + 1) * 512])
        wo = [const.tile([128, D], f16, tag=f"wo{j}", name=f"wo{j}") for j in range(4)]
        for j in range(4):
            dma_in(wo[j][:], wo_d[j * 128 : (j + 1) * 128, :])
        bo_t = const.tile([128, D], f32, tag="bob", name="bob")
        dma_in(bo_t[:], bo_d[:])

        # ---- persistent activations ----
        QT = [[actp.tile([128, 512], f16, tag=f"qt{ct}_{sc}", name=f"qt{ct}_{sc}") for sc in range(4)]
              for ct in range(4)]
        KT = [[actp.tile([128, 512], f16, tag=f"kt{ct}_{sc}", name=f"kt{ct}_{sc}") for sc in range(4)]
              for ct in range(4)]
        V = [actp.tile([128, 8 * 65], f16, tag=f"v{st}", name=f"v{st}") for st in range(16)]
        OTn = [[actp.tile([128, 512], f16, tag=f"otn{hp}_{qc}", name=f"otn{hp}_{qc}") for qc in range(4)]
               for hp in range(4)]

        # ---- PE filler quanta -------------------------------------------
        # The attention kt loop leaves the PE ~40% idle (waiting on ACT exp).
        # Those slots are filled by popping emission closures from a queue:
        # the previous chunk's out-projection groups and the next chunk's
        # QKV GEMM groups.
        filler = []

        def pop_filler():
            if filler:
                filler.pop(0)()

        def drain_filler():
            while filler:
                filler.pop(0)()

        def emit_outproj_group(qc, stl, oc):
            st = 4 * qc + stl
            sl = slice(stl * 128, (stl + 1) * 128)
            ocs = slice(oc * 512, (oc + 1) * 512)
            # hpp=3 first: the group's first matmul then depends on the LAST
            # head-pair's normalization, so the scheduler cannot hoist it
            # into the middle of the kt stream where its semaphore wait
            # would block the whole PE queue.
            yp = ps_p.tile([128, 512], f32, tag="p512", name="p512")
            for hpp in (3, 2, 1, 0):
                nc.tensor.matmul(yp[:], OTn[hpp][qc][:, sl],
                                 wo[hpp][:, ocs],
                                 start=(hpp == 3), stop=(hpp == 0),
                                 skip_group_check=True)
            ysb = work.tile([128, 512], f16, tag="ysb", name="ysb")
            nc.vector.scalar_tensor_tensor(ysb[:], yp[:], 1.0,
                                           bo_t[:, ocs], mult, add)
            nc.sync.dma_start(y_d[st * 128 : (st + 1) * 128, ocs], ysb[:])

        def queue_outproj(qc):
            for stl in range(4):
                for oc in range(2):
                    filler.append(
                        lambda qc=qc, stl=stl, oc=oc: emit_outproj_group(qc, stl, oc))

        def emit_kt_group(ct, sc):
            cs = slice(ct * 128, (ct + 1) * 128)
            p = ps_p.tile([128, 512], f32, tag="p512", name="p512")
            for d in range(8):
                nc.tensor.matmul(p[:], wk[d][:, cs], xt[d][sc][:],
                                 start=(d == 0), stop=(d == 7),
                                 skip_group_check=True)
            nc.vector.tensor_scalar_add(KT[ct][sc][:], p[:], bk[ct][:])

        def emit_qt_group(ct, sc):
            cs = slice(ct * 128, (ct + 1) * 128)
            p = ps_p.tile([128, 512], f32, tag="p512", name="p512")
            for d in range(8):
                nc.tensor.matmul(p[:], wq[d][:, cs], xt[d][sc][:],
                                 start=(d == 0), stop=(d == 7),
                                 skip_group_check=True)
            nc.vector.tensor_scalar_add(QT[ct][sc][:], p[:], bq[ct][:])

        def emit_v_group(stl, sc):
            st = 4 * sc + stl
            ts = slice(stl * 128, (stl + 1) * 128)
            p = ps_p.tile([128, 512], f32, tag="p512", name="p512")
            for d in range(8):
                nc.tensor.matmul(p[:], xt[d][sc][:, ts], wv[d][:, :],
                                 start=(d == 0), stop=(d == 7),
                                 skip_group_check=True)
            v3 = V[st][:].rearrange("p (h e) -> p h e", e=65)
            nc.vector.scalar_tensor_tensor(
                v3[:, :, 0:64],
                p[:].rearrange("p (h e) -> p h e", e=64),
                1.0,
                bv_t[:].rearrange("p (h e) -> p h e", e=64),
                mult, add,
            )
            nc.vector.memset(v3[:, :, 64:65], 1.0)

        def queue_qkv(sc):
            # K first (S-matmul stationary), then V, then Q.
            for ct in range(4):
                filler.append(lambda ct=ct, sc=sc: emit_kt_group(ct, sc))
            for stl in range(4):
                filler.append(lambda stl=stl, sc=sc: emit_v_group(stl, sc))
            for ct in range(4):
                filler.append(lambda ct=ct, sc=sc: emit_qt_group(ct, sc))

        # ---- attention software pipeline state (spans hp/chunk boundaries) --
        pend = []    # entries awaiting their AV matmuls
        flip = [0]

        def emit_norm(e):
            # softmax normalization: evacuate av PSUM to SBUF right away
            # (frees the PSUM pool), then normalize from SBUF off the
            # critical path. Row 64 of av = softmax denominator.
            hp, qc = e["hp"], e["qc"]
            avsb = normp.tile([65, 1024], f32, tag="avsb", name="avsb")
            nc.vector.tensor_copy(avsb[:, 0:512], e["av0"][:])
            nc.vector.tensor_copy(avsb[:, 512:1024], e["av1"][:])
            # 1/d = exp(-ln(d)) on ACT (both fns share one table set): a
            # [1,N] reciprocal on the DVE (3.3us, single lane) sits on the
            # OTn dependence chain and stalls the PE queue behind it.
            r = work.tile([1, 1024], f32, tag="r", name="r")
            lnt = work.tile([1, 1024], f32, tag="lnt", name="lnt")
            nc.scalar.activation(lnt[:], avsb[64:65, :], Ln)
            nc.scalar.activation(r[:], lnt[:], Exp, scale=-1.0)
            rb0 = work.tile([64, 512], f32, tag="rb", name="rb0")
            nc.gpsimd.partition_broadcast(rb0[:], r[0:1, 0:512], channels=64)
            nc.vector.tensor_mul(OTn[hp][qc][0:64, :], avsb[0:64, 0:512],
                                 rb0[:])
            rb1 = work.tile([64, 512], f32, tag="rb", name="rb1")
            nc.gpsimd.partition_broadcast(rb1[:], r[0:1, 512:1024], channels=64)
            nc.vector.tensor_mul(OTn[hp][qc][64:128, :], avsb[0:64, 512:1024],
                                 rb1[:])

        def flush_av():
            e = pend.pop(0)
            d = e["delta"]
            nc.tensor.matmul(
                e["av0"][:, d:512], V[e["kt"]][:, e["h0"] * 65 : e["h0"] * 65 + 65],
                e["pt"][:, d:512],
                start=e["first"], stop=e["last"], skip_group_check=True)
            nc.tensor.matmul(
                e["av1"][:, d:512], V[e["kt"]][:, e["h1"] * 65 : e["h1"] * 65 + 65],
                e["pt"][:, 512 + d : 1024],
                start=e["first"], stop=e["last"], skip_group_check=True)
            if e["last"]:
                emit_norm(e)

        # ---- chunk 0 QKV: only what hp0's attention needs runs up front
        # (KT/QT ct=0 + all V); the other head-pairs' K/Q projections are
        # queued as filler consumed during earlier head-pairs' kt loops.
        emit_kt_group(0, 0)
        emit_qt_group(0, 0)
        for stl in range(4):
            emit_v_group(stl, 0)

        for sc in range(4):
            # Queue this chunk's deferred PE work: previous chunk's
            # out-projection, then the NEXT chunk's QKV projections.
            if sc == 0:
                for ct in range(1, 4):
                    filler.append(lambda ct=ct: emit_kt_group(ct, 0))
                    filler.append(lambda ct=ct: emit_qt_group(ct, 0))
            # QKV(sc+1) first: the out-projection quanta need ALL of chunk
            # sc-1's OTn tiles, whose last normalization lands a few us into
            # this chunk -- popping them later avoids a PE stall. In the last
            # chunk (no QKV left, PE otherwise starved) outproj goes first.
            if sc < 3:
                queue_qkv(sc + 1)
            if sc > 0:
                queue_outproj(sc - 1)

            # attention for query chunk qc = sc, head pairs interleaved so the
            # even head's K=64 matmuls (rows 0-63) and the odd head's (rows
            # 64-127) run concurrently in the PE array. The S->exp->AV
            # software pipeline (pend) runs FLAT across head-pair and chunk
            # boundaries so the ACT exp stream never drains at a boundary.
            qc = sc
            nkt = 4 * (qc + 1)
            for hp in range(4):
                h0, h1 = 2 * hp, 2 * hp + 1
                av0 = ps_p.tile([65, 512], f32, tag="p512", name="av0")
                av1 = ps_p.tile([65, 512], f32, tag="p512", name="av1")

                for kt in range(nkt):
                    # merged S^T tile: cols 0-511 head h0, 512-1023 head h1.
                    # Diagonal-straddling blocks: cols < delta are fully
                    # masked -> not computed at all; the 128-wide strip
                    # [delta, delta+128) gets the -BIG ramp added.
                    diag = kt >= 4 * qc
                    delta = 128 * kt - 512 * qc if diag else 0
                    sp = ps_s.tile([128, 1024], f32, tag="s2", name="sp")
                    kcol = slice((kt % 4) * 128, (kt % 4) * 128 + 128)
                    nc.tensor.matmul(
                        sp[:, delta:512], KT[hp][kt // 4][0:64, kcol],
                        QT[hp][qc][0:64, delta:512],
                        start=True, stop=not diag, skip_group_check=True)
                    nc.tensor.matmul(
                        sp[:, 512 + delta : 1024], KT[hp][kt // 4][64:128, kcol],
                        QT[hp][qc][64:128, delta:512],
                        start=True, stop=not diag, skip_group_check=True)
                    if diag:
                        rsl = slice(384, 512)
                        nc.tensor.matmul(sp[:, delta : delta + 128], U_t[:],
                                         R_t[:, rsl],
                                         start=False, stop=True,
                                         skip_group_check=True)
                        nc.tensor.matmul(sp[:, 512 + delta : 512 + delta + 128],
                                         U_t[:], R_t[:, rsl],
                                         start=False, stop=True,
                                         skip_group_check=True)
                    pt = ptp.tile([128, 1024], f16, tag="pt", name="pt")
                    sp3 = sp[:].rearrange("p (h q) -> p h q", h=2)
                    pt3 = pt[:].rearrange("p (h q) -> p h q", h=2)
                    nc.scalar.activation(pt3[:, :, delta:512],
                                         sp3[:, :, delta:512], Exp, scale=SCALE)
                    pend.append(dict(kt=kt, pt=pt, delta=delta, av0=av0,
                                     av1=av1, h0=h0, h1=h1, hp=hp, qc=qc,
                                     first=(kt == 0), last=(kt == nkt - 1)))
                    pop_filler()
                    # AVs trail the exp stream by >=2 entries and are emitted
                    # in batches of 2: fewer S<->AV stationary switches means
                    # fewer un-hidden LDWEIGHTS on the PE.
                    flip[0] ^= 1
                    if flip[0] == 0:
                        while len(pend) > 2:
                            flush_av()
            drain_filler()
        while pend:
            flush_av()
        queue_outproj(3)
        drain_filler()

    nc.compile()
    bacc.get_activation_tables = orig_tables
    return nc


def _host_inputs(x, w_qkv, b_qkv, w_o, b_o):
    """Per-core input dicts implementing the sharding + layout prep."""
    U = np.zeros((128, 128), np.float16)
    for c in range(128):
        U[c, c:] = 1.0
    R = np.zeros((128, 896), np.float16)
    for c in range(128):
        R[c, : c + 384] = -BIG

    in_maps = []
    for c in range(N_CORES):
        b = c // 2
        hs = (c % 2) * HPC
        cols = slice(hs * DH, (hs + HPC) * DH)
        in_maps.append({
            "xT": np.ascontiguousarray(x[b].T).astype(np.float16),
            "wq": w_qkv[:, cols].astype(np.float16),
            "wk": w_qkv[:, D:][:, cols].astype(np.float16),
            "wv": w_qkv[:, 2 * D:][:, cols].astype(np.float16),
            "wo": w_o[hs * DH : (hs + HPC) * DH, :].astype(np.float16),
            "bq": b_qkv[cols].reshape(CH, 1).astype(np.float32),
            "bk": b_qkv[D:][cols].reshape(CH, 1).astype(np.float32),
            "bvb": np.tile(b_qkv[2 * D:][cols].astype(np.float32), (128, 1)),
            "bob": np.tile(b_o.astype(np.float32), (128, 1)),
            "U": U,
            "R": R,
        })
    return in_maps


def kernel(x, w_qkv, b_qkv, w_o, b_o):
    global _cached
    from concourse.bass_utils import run_bass_kernel_spmd

    x = np.asarray(x)
    w_qkv = np.asarray(w_qkv)
    b_qkv = np.asarray(b_qkv)
    w_o = np.asarray(w_o)
    b_o = np.asarray(b_o)

    if _cached is None:
        _cached = _build_program()
    nc = _cached

    in_maps = _host_inputs(x, w_qkv, b_qkv, w_o, b_o)
    res = run_bass_kernel_spmd(nc, in_maps, list(range(N_CORES)))

    out = np.empty((B, N, D), np.float32)
    for b in range(B):
        out[b] = (res.results[2 * b]["y"].astype(np.float32)
                  + res.results[2 * b + 1]["y"].astype(np.float32))
    return out


# revision 23
# speedup vs baseline: 1.3732x; 1.0829x over previous
"""Causal self-attention on 8 Trainium2 NeuronCores.

Sharding: core c handles batch b = c//2 and heads [(c%2)*8, (c%2)*8+8).
Each core computes the full QKV projection for its head slice, causal
flash-style attention, and the row-parallel w_o partial product. The two
partials per batch are summed on the host (no device collectives).

All PE matmuls run in fp16 (1 cycle/row) with fp32 PSUM accumulation.
Feature-major layouts throughout:
  x^T [D, N]        (host pre-transposed)
  Q^T, K^T [ch, N]  (from GEMM with W stationary, x^T moving)
  V [N, ch] + ones  (from GEMM with x^T stationary, W moving)
  S^T [k, q] = K^T_tile.T @ Q^T  -> exp -> P^T [k, q]
  O^T [ch, q] = (V|1).T @ P^T    (row 64 = softmax denominator)
  y = O^T_norm.T @ W_o           (accumulated over ch tiles)

Causal masking: diagonal-straddling S^T blocks only compute columns
>= delta (cols below are fully masked); the 128-wide partial strip gets
-BIG * max(k - u, 0) added via an extra accumulating matmul so exp()
underflows to exact zeros.

The attention kt loop is ACT(exp)-bound; PE idle inside it is filled by
interleaving the previous chunk's out-projection and the next chunk's
QKV matmuls as filler quanta.
"""

import numpy as np

B, N, D, H = 4, 2048, 1024, 16
DH = 64
N_CORES = 8
HPC = 8            # heads per core
CH = HPC * DH      # 512 channels per core
SCALE = 1.0 / 8.0  # 1/sqrt(DH)
BIG = 280.0        # SCALE*BIG = 35 >> max |S/8|, exp underflows to 0

_cached = None


def _build_program():
    from contextlib import ExitStack

    import concourse.tile as tile
    from concourse import bacc, mybir

    f16 = mybir.dt.float16
    f32 = mybir.dt.float32
    Exp = mybir.ActivationFunctionType.Exp
    Ln = mybir.ActivationFunctionType.Ln
    mult = mybir.AluOpType.mult
    add = mybir.AluOpType.add

    # The kernel uses both Exp (softmax) and Ln (reciprocal-via-exp(-ln)).
    # The table-load placement pass assigns each activation the first set
    # containing its function, which thrashes ~27 ACT_TABLE_LOADs between
    # `exp_and_others` and `natural_log`. Restrict Exp/Ln to the combined
    # `natural_log_exp_and_others` set (names/order unchanged, so the
    # act_func_set_id indexing stays valid) -> exactly one load.
    orig_tables = bacc.get_activation_tables

    def _patched_tables(arch):
        t = dict(orig_tables(arch))
        for name, fns in t.items():
            if name != "natural_log_exp_and_others":
                t[name] = {
                    f for f in fns
                    if f not in (mybir.ActivationFunctionType.Exp,
                                 mybir.ActivationFunctionType.Ln)
                }
        return t

    bacc.get_activation_tables = _patched_tables

    nc = bacc.Bacc(
        "TRN2", target_bir_lowering=False, debug=False, num_devices=N_CORES
    )

    xT_d = nc.dram_tensor("xT", [D, N], f16, kind="ExternalInput").ap()
    wq_d = nc.dram_tensor("wq", [D, CH], f16, kind="ExternalInput").ap()
    wk_d = nc.dram_tensor("wk", [D, CH], f16, kind="ExternalInput").ap()
    wv_d = nc.dram_tensor("wv", [D, CH], f16, kind="ExternalInput").ap()
    wo_d = nc.dram_tensor("wo", [CH, D], f16, kind="ExternalInput").ap()
    bq_d = nc.dram_tensor("bq", [CH, 1], f32, kind="ExternalInput").ap()
    bk_d = nc.dram_tensor("bk", [CH, 1], f32, kind="ExternalInput").ap()
    bv_d = nc.dram_tensor("bvb", [128, CH], f32, kind="ExternalInput").ap()
    bo_d = nc.dram_tensor("bob", [128, D], f32, kind="ExternalInput").ap()
    U_d = nc.dram_tensor("U", [128, 128], f16, kind="ExternalInput").ap()
    R_d = nc.dram_tensor("R", [128, 896], f16, kind="ExternalInput").ap()
    y_d = nc.dram_tensor("y", [N, D], f16, kind="ExternalOutput").ap()

    with tile.TileContext(nc) as tc, ExitStack() as ctx:
        const = ctx.enter_context(tc.tile_pool(name="const", bufs=1))
        actp = ctx.enter_context(tc.tile_pool(name="actp", bufs=1))
        work = ctx.enter_context(tc.tile_pool(name="work", bufs=3))
        ptp = ctx.enter_context(tc.tile_pool(name="ptp", bufs=5))
        normp = ctx.enter_context(tc.tile_pool(name="normp", bufs=2))
        ps_s = ctx.enter_context(tc.tile_pool(name="ps_s", bufs=2, space="PSUM"))
        ps_p = ctx.enter_context(tc.tile_pool(name="ps_p", bufs=4, space="PSUM"))

        # ---- constants / weights into SBUF ----
        # K-weights + first seq-chunk of x first so the K^T GEMM starts ASAP.
        wq = [const.tile([128, CH], f16, tag=f"wq{i}", name=f"wq{i}") for i in range(8)]
        wk = [const.tile([128, CH], f16, tag=f"wk{i}", name=f"wk{i}") for i in range(8)]
        wv = [const.tile([128, CH], f16, tag=f"wv{i}", name=f"wv{i}") for i in range(8)]
        xt = [[const.tile([128, 512], f16, tag=f"xt{i}_{sc}", name=f"xt{i}_{sc}")
               for sc in range(4)] for i in range(8)]
        # Round-robin input DMAs across engine queues so the 2D row-descriptor
        # processing runs in parallel; first-needed first. The first wave
        # (wk + first x chunk) additionally uses the vector/scalar queues,
        # which are compute-idle until the first GEMM finishes.
        first_engs = [nc.sync, nc.gpsimd, nc.scalar]
        engs = [nc.sync, nc.gpsimd]
        _ei = [0]

        def dma_first(dst, src):
            first_engs[_ei[0] % len(first_engs)].dma_start(dst, src)
            _ei[0] += 1

        def dma_in(dst, src):
            engs[_ei[0] % len(engs)].dma_start(dst, src)
            _ei[0] += 1

        # wave 1: just the ct=0 column slices of wk/wq + x chunk 0 + wv,
        # so the first KT/QT/V GEMMs can start after ~2.5MB instead of ~5MB.
        for i in range(8):
            dma_first(wk[i][:, 0:128], wk_d[i * 128 : (i + 1) * 128, 0:128])
            dma_first(xt[i][0][:], xT_d[i * 128 : (i + 1) * 128, 0:512])
        bq = [const.tile([128, 1], f32, tag=f"bq{j}", name=f"bq{j}") for j in range(4)]
        bk = [const.tile([128, 1], f32, tag=f"bk{j}", name=f"bk{j}") for j in range(4)]
        dma_first(bk[0][:], bk_d[0:128, :])
        dma_first(bq[0][:], bq_d[0:128, :])
        bv_t = const.tile([128, CH], f32, tag="bvb", name="bvb")
        dma_first(bv_t[:], bv_d[:])
        for i in range(8):
            dma_first(wq[i][:, 0:128], wq_d[i * 128 : (i + 1) * 128, 0:128])
            dma_first(wv[i][:], wv_d[i * 128 : (i + 1) * 128, :])
        # U/R are tiny (256KB) and attention(0)'s diagonal masking needs
        # them right after the first QKV groups -> load before wave 2.
        U_t = const.tile([128, 128], f16, tag="U", name="Ut")
        dma_first(U_t[:], U_d[:])
        R_t = const.tile([128, 896], f16, tag="R", name="Rt")
        dma_first(R_t[:], R_d[:])
        # wave 2: remainders and later chunks.
        for i in range(8):
            dma_in(wk[i][:, 128:CH], wk_d[i * 128 : (i + 1) * 128, 128:CH])
            dma_in(wq[i][:, 128:CH], wq_d[i * 128 : (i + 1) * 128, 128:CH])
        for j in range(1, 4):
            dma_in(bq[j][:], bq_d[j * 128 : (j + 1) * 128, :])
            dma_in(bk[j][:], bk_d[j * 128 : (j + 1) * 128, :])
        for sc in range(1, 4):
            for i in range(8):
                dma_in(xt[i][sc][:],
                       xT_d[i * 128 : (i + 1) * 128, sc * 512 : (sc + 1) * 512])
        wo = [const.tile([128, D], f16, tag=f"wo{j}", name=f"wo{j}") for j in range(4)]
        for j in range(4):
            dma_in(wo[j][:], wo_d[j * 128 : (j + 1) * 128, :])
        bo_t = const.tile([128, D], f32, tag="bob", name="bob")
        dma_in(bo_t[:], bo_d[:])

        # ---- persistent activations ----
        QT = [[actp.tile([128, 512], f16, tag=f"qt{ct}_{sc}", name=f"qt{ct}_{sc}") for sc in range(4)]
              for ct in range(4)]
        KT = [[actp.tile([128, 512], f16, tag=f"kt{ct}_{sc}", name=f"kt{ct}_{sc}") for sc in range(4)]
              for ct in range(4)]
        V = [actp.tile([128, 8 * 65], f16, tag=f"v{st}", name=f"v{st}") for st in range(16)]
        OTn = [[actp.tile([128, 512], f16, tag=f"otn{hp}_{qc}", name=f"otn{hp}_{qc}") for qc in range(4)]
               for hp in range(4)]

        # ---- PE filler quanta -------------------------------------------
        # The attention kt loop leaves the PE ~40% idle (waiting on ACT exp).
        # Those slots are filled by popping emission closures from a queue:
        # the previous chunk's out-projection groups and the next chunk's
        # QKV GEMM groups.
        filler = []

        def pop_filler():
            if filler:
                filler.pop(0)()

        def drain_filler():
            while filler:
                filler.pop(0)()

        def emit_outproj_group(qc, stl, oc):
            st = 4 * qc + stl
            sl = slice(stl * 128, (stl + 1) * 128)
            ocs = slice(oc * 512, (oc + 1) * 512)
            # hpp=3 first: the group's first matmul then depends on the LAST
            # head-pair's normalization, so the scheduler cannot hoist it
            # into the middle of the kt stream where its semaphore wait
            # would block the whole PE queue.
            yp = ps_p.tile([128, 512], f32, tag="p512", name="p512")
            for hpp in (3, 2, 1, 0):
                nc.tensor.matmul(yp[:], OTn[hpp][qc][:, sl],
                                 wo[hpp][:, ocs],
                                 start=(hpp == 3), stop=(hpp == 0),
                                 skip_group_check=True)
            ysb = work.tile([128, 512], f16, tag="ysb", name="ysb")
            nc.vector.scalar_tensor_tensor(ysb[:], yp[:], 1.0,
                                           bo_t[:, ocs], mult, add)
            nc.sync.dma_start(y_d[st * 128 : (st + 1) * 128, ocs], ysb[:])

        def queue_outproj(qc):
            for stl in range(4):
                for oc in range(2):
                    filler.append(
                        lambda qc=qc, stl=stl, oc=oc: emit_outproj_group(qc, stl, oc))

        def emit_kt_group(ct, sc):
            cs = slice(ct * 128, (ct + 1) * 128)
            p = ps_p.tile([128, 512], f32, tag="p512", name="p512")
            for d in range(8):
                nc.tensor.matmul(p[:], wk[d][:, cs], xt[d][sc][:],
                                 start=(d == 0), stop=(d == 7),
                                 skip_group_check=True)
            nc.vector.tensor_scalar_add(KT[ct][sc][:], p[:], bk[ct][:])

        def emit_qt_group(ct, sc):
            cs = slice(ct * 128, (ct + 1) * 128)
            p = ps_p.tile([128, 512], f32, tag="p512", name="p512")
            for d in range(8):
                nc.tensor.matmul(p[:], wq[d][:, cs], xt[d][sc][:],
                                 start=(d == 0), stop=(d == 7),
                                 skip_group_check=True)
            nc.vector.tensor_scalar_add(QT[ct][sc][:], p[:], bq[ct][:])

        def emit_v_group(stl, sc):
            st = 4 * sc + stl
            ts = slice(stl * 128, (stl + 1) * 128)
            p = ps_p.tile([128, 512], f32, tag="p512", name="p512")
            for d in range(8):
                nc.tensor.matmul(p[:], xt[d][sc][:, ts], wv[d][:, :],
                                 start=(d == 0), stop=(d == 7),
                                 skip_group_check=True)
            v3 = V[st][:].rearrange("p (h e) -> p h e", e=65)
            nc.vector.scalar_tensor_tensor(
                v3[:, :, 0:64],
                p[:].rearrange("p (h e) -> p h e", e=64),
                1.0,
                bv_t[:].rearrange("p (h e) -> p h e", e=64),
                mult, add,
            )
            nc.vector.memset(v3[:, :, 64:65], 1.0)

        def queue_qkv(sc):
            # K first (S-matmul stationary), then V, then Q.
            for ct in range(4):
                filler.append(lambda ct=ct, sc=sc: emit_kt_group(ct, sc))
            for stl in range(4):
                filler.append(lambda stl=stl, sc=sc: emit_v_group(stl, sc))
            for ct in range(4):
                filler.append(lambda ct=ct, sc=sc: emit_qt_group(ct, sc))

        # ---- attention software pipeline state (spans hp/chunk boundaries) --
        pend = []    # entries awaiting their AV matmuls
        flip = [0]

        def emit_norm(e):
            # softmax normalization: evacuate av PSUM to SBUF right away
            # (frees the PSUM pool), then normalize from SBUF off the
            # critical path. Row 64 of av = softmax denominator.
            hp, qc = e["hp"], e["qc"]
            avsb = normp.tile([65, 1024], f32, tag="avsb", name="avsb")
            nc.vector.tensor_copy(avsb[:, 0:512], e["av0"][:])
            nc.vector.tensor_copy(avsb[:, 512:1024], e["av1"][:])
            # 1/d = exp(-ln(d)) on ACT (both fns share one table set): a
            # [1,N] reciprocal on the DVE (3.3us, single lane) sits on the
            # OTn dependence chain and stalls the PE queue behind it.
            r = work.tile([1, 1024], f32, tag="r", name="r")
            lnt = work.tile([1, 1024], f32, tag="lnt", name="lnt")
            nc.scalar.activation(lnt[:], avsb[64:65, :], Ln)
            nc.scalar.activation(r[:], lnt[:], Exp, scale=-1.0)
            rb0 = work.tile([64, 512], f32, tag="rb", name="rb0")
            nc.gpsimd.partition_broadcast(rb0[:], r[0:1, 0:512], channels=64)
            nc.vector.tensor_mul(OTn[hp][qc][0:64, :], avsb[0:64, 0:512],
                                 rb0[:])
            rb1 = work.tile([64, 512], f32, tag="rb", name="rb1")
            nc.gpsimd.partition_broadcast(rb1[:], r[0:1, 512:1024], channels=64)
            nc.vector.tensor_mul(OTn[hp][qc][64:128, :], avsb[0:64, 512:1024],
                                 rb1[:])

        def flush_av():
            e = pend.pop(0)
            d = e["delta"]
            nc.tensor.matmul(
                e["av0"][:, d:512], V[e["kt"]][:, e["h0"] * 65 : e["h0"] * 65 + 65],
                e["pt"][:, d:512],
                start=e["first"], stop=e["last"], skip_group_check=True)
            nc.tensor.matmul(
                e["av1"][:, d:512], V[e["kt"]][:, e["h1"] * 65 : e["h1"] * 65 + 65],
                e["pt"][:, 512 + d : 1024],
                start=e["first"], stop=e["last"], skip_group_check=True)
            if e["last"]:
                emit_norm(e)

        # ---- chunk 0 QKV: only what hp0's attention needs runs up front
        # (KT/QT ct=0 + all V); the other head-pairs' K/Q projections are
        # queued as filler consumed during earlier head-pairs' kt loops.
        emit_kt_group(0, 0)
        emit_qt_group(0, 0)
        for stl in range(4):
            emit_v_group(stl, 0)

        # Chunk 3's attention is ACT(exp)-bound while chunks 0-2 are
        # PE-bound: shift chunk 3's first head-pair into chunk 2's emission
        # window (its QKV(3) inputs are produced by chunk 2's fillers well
        # before the appended item runs).
        plan = [
            [(0, hp) for hp in range(4)],
            [(1, hp) for hp in range(4)],
            [(2, hp) for hp in range(4)] + [(3, 0)],
            [(3, hp) for hp in (1, 2, 3)],
        ]
        for sc in range(4):
            # Queue this chunk's deferred PE work: previous chunk's
            # out-projection, then the NEXT chunk's QKV projections.
            if sc == 0:
                for ct in range(1, 4):
                    filler.append(lambda ct=ct: emit_kt_group(ct, 0))
                    filler.append(lambda ct=ct: emit_qt_group(ct, 0))
            # QKV(sc+1) first: the out-projection quanta need ALL of chunk
            # sc-1's OTn tiles, whose last normalization lands a few us into
            # this chunk -- popping them later avoids a PE stall. In the last
            # chunk (no QKV left, PE otherwise starved) outproj goes first.
            if sc < 3:
                queue_qkv(sc + 1)
            if sc > 0:
                queue_outproj(sc - 1)

            # Pace filler pops evenly over this chunk's kt slots (Bresenham)
            # instead of front-loading 1-per-kt: front-loading leaves the
            # late kt slots running at ACT rate with the PE starved. Chunk 0
            # keeps 1-per-kt: its first fillers are chunk-0 K/Q projections
            # that later head-pairs in the SAME chunk depend on.
            nslots = sum(4 * (q + 1) for q, _ in plan[sc])
            nfill0 = len(filler)
            state = {"slot": 0, "popped": 0}

            def paced_pop(sc=sc, nslots=nslots, nfill0=nfill0, state=state):
                state["slot"] += 1
                if sc == 0:
                    pop_filler()
                    state["popped"] += 1
                    return
                while (filler
                       and state["popped"] * nslots < state["slot"] * nfill0):
                    pop_filler()
                    state["popped"] += 1

            # attention: head pairs interleaved so the even head's K=64
            # matmuls (rows 0-63) and the odd head's (rows 64-127) run
            # concurrently in the PE array. The S->exp->AV software pipeline
            # (pend) runs FLAT across head-pair and chunk boundaries so the
            # ACT exp stream never drains at a boundary.
            for qc, hp in plan[sc]:
                nkt = 4 * (qc + 1)
                h0, h1 = 2 * hp, 2 * hp + 1
                av0 = ps_p.tile([65, 512], f32, tag="p512", name="av0")
                av1 = ps_p.tile([65, 512], f32, tag="p512", name="av1")

                for kt in range(nkt):
                    # merged S^T tile: cols 0-511 head h0, 512-1023 head h1.
                    # Diagonal-straddling blocks: cols < delta are fully
                    # masked -> not computed at all; the 128-wide strip
                    # [delta, delta+128) gets the -BIG ramp added.
                    diag = kt >= 4 * qc
                    delta = 128 * kt - 512 * qc if diag else 0
                    sp = ps_s.tile([128, 1024], f32, tag="s2", name="sp")
                    kcol = slice((kt % 4) * 128, (kt % 4) * 128 + 128)
                    nc.tensor.matmul(
                        sp[:, delta:512], KT[hp][kt // 4][0:64, kcol],
                        QT[hp][qc][0:64, delta:512],
                        start=True, stop=not diag, skip_group_check=True)
                    nc.tensor.matmul(
                        sp[:, 512 + delta : 1024], KT[hp][kt // 4][64:128, kcol],
                        QT[hp][qc][64:128, delta:512],
                        start=True, stop=not diag, skip_group_check=True)
                    if diag:
                        rsl = slice(384, 512)
                        nc.tensor.matmul(sp[:, delta : delta + 128], U_t[:],
                                         R_t[:, rsl],
                                         start=False, stop=True,
                                         skip_group_check=True)
                        nc.tensor.matmul(sp[:, 512 + delta : 512 + delta + 128],
                                         U_t[:], R_t[:, rsl],
                                         start=False, stop=True,
                                         skip_group_check=True)
                    pt = ptp.tile([128, 1024], f16, tag="pt", name="pt")
                    sp3 = sp[:].rearrange("p (h q) -> p h q", h=2)
                    pt3 = pt[:].rearrange("p (h q) -> p h q", h=2)
                    nc.scalar.activation(pt3[:, :, delta:512],
                                         sp3[:, :, delta:512], Exp, scale=SCALE)
                    pend.append(dict(kt=kt, pt=pt, delta=delta, av0=av0,
                                     av1=av1, h0=h0, h1=h1, hp=hp, qc=qc,
                                     first=(kt == 0), last=(kt == nkt - 1)))
                    paced_pop()
                    # AVs trail the exp stream by >=2 entries and are emitted
                    # in batches of 2: fewer S<->AV stationary switches means
                    # fewer un-hidden LDWEIGHTS on the PE.
                    flip[0] ^= 1
                    if flip[0] == 0:
                        while len(pend) > 2:
                            flush_av()
            drain_filler()
        while pend:
            flush_av()
        queue_outproj(3)
        drain_filler()

    nc.compile()
    bacc.get_activation_tables = orig_tables
    return nc


def _host_inputs(x, w_qkv, b_qkv, w_o, b_o):
    """Per-core input dicts implementing the sharding + layout prep."""
    U = np.zeros((128, 128), np.float16)
    for c in range(128):
        U[c, c:] = 1.0
    R = np.zeros((128, 896), np.float16)
    for c in range(128):
        R[c, : c + 384] = -BIG

    in_maps = []
    for c in range(N_CORES):
        b = c // 2
        hs = (c % 2) * HPC
        cols = slice(hs * DH, (hs + HPC) * DH)
        in_maps.append({
            "xT": np.ascontiguousarray(x[b].T).astype(np.float16),
            "wq": w_qkv[:, cols].astype(np.float16),
            "wk": w_qkv[:, D:][:, cols].astype(np.float16),
            "wv": w_qkv[:, 2 * D:][:, cols].astype(np.float16),
            "wo": w_o[hs * DH : (hs + HPC) * DH, :].astype(np.float16),
            "bq": b_qkv[cols].reshape(CH, 1).astype(np.float32),
            "bk": b_qkv[D:][cols].reshape(CH, 1).astype(np.float32),
            "bvb": np.tile(b_qkv[2 * D:][cols].astype(np.float32), (128, 1)),
            "bob": np.tile(b_o.astype(np.float32), (128, 1)),
            "U": U,
            "R": R,
        })
    return in_maps


def kernel(x, w_qkv, b_qkv, w_o, b_o):
    global _cached
    from concourse.bass_utils import run_bass_kernel_spmd

    x = np.asarray(x)
    w_qkv = np.asarray(w_qkv)
    b_qkv = np.asarray(b_qkv)
    w_o = np.asarray(w_o)
    b_o = np.asarray(b_o)

    if _cached is None:
        _cached = _build_program()
    nc = _cached

    in_maps = _host_inputs(x, w_qkv, b_qkv, w_o, b_o)
    res = run_bass_kernel_spmd(nc, in_maps, list(range(N_CORES)))

    out = np.empty((B, N, D), np.float32)
    for b in range(B):
        out[b] = (res.results[2 * b]["y"].astype(np.float32)
                  + res.results[2 * b + 1]["y"].astype(np.float32))
    return out


# revision 24
# speedup vs baseline: 1.4048x; 1.0230x over previous
"""Causal self-attention on 8 Trainium2 NeuronCores.

Sharding: core c handles batch b = c//2 and heads [(c%2)*8, (c%2)*8+8).
Each core computes the full QKV projection for its head slice, causal
flash-style attention, and the row-parallel w_o partial product. The two
partials per batch are summed on the host (no device collectives).

All PE matmuls run in fp16 (1 cycle/row) with fp32 PSUM accumulation.
Feature-major layouts throughout:
  x^T [D, N]        (host pre-transposed)
  Q^T, K^T [ch, N]  (from GEMM with W stationary, x^T moving)
  V [N, ch] + ones  (from GEMM with x^T stationary, W moving)
  S^T [k, q] = K^T_tile.T @ Q^T  -> exp -> P^T [k, q]
  O^T [ch, q] = (V|1).T @ P^T    (row 64 = softmax denominator)
  y = O^T_norm.T @ W_o           (accumulated over ch tiles)

Causal masking: diagonal-straddling S^T blocks only compute columns
>= delta (cols below are fully masked); the 128-wide partial strip gets
-BIG * max(k - u, 0) added via an extra accumulating matmul so exp()
underflows to exact zeros.

The attention kt loop is ACT(exp)-bound; PE idle inside it is filled by
interleaving the previous chunk's out-projection and the next chunk's
QKV matmuls as filler quanta.
"""

import numpy as np

B, N, D, H = 4, 2048, 1024, 16
DH = 64
N_CORES = 8
HPC = 8            # heads per core
CH = HPC * DH      # 512 channels per core
SCALE = 1.0 / 8.0  # 1/sqrt(DH)
BIG = 280.0        # SCALE*BIG = 35 >> max |S/8|, exp underflows to 0

_cached = None


def _build_program():
    from contextlib import ExitStack

    import concourse.tile as tile
    from concourse import bacc, mybir

    f16 = mybir.dt.float16
    f32 = mybir.dt.float32
    Exp = mybir.ActivationFunctionType.Exp
    Ln = mybir.ActivationFunctionType.Ln
    mult = mybir.AluOpType.mult
    add = mybir.AluOpType.add

    # The kernel uses both Exp (softmax) and Ln (reciprocal-via-exp(-ln)).
    # The table-load placement pass assigns each activation the first set
    # containing its function, which thrashes ~27 ACT_TABLE_LOADs between
    # `exp_and_others` and `natural_log`. Restrict Exp/Ln to the combined
    # `natural_log_exp_and_others` set (names/order unchanged, so the
    # act_func_set_id indexing stays valid) -> exactly one load.
    orig_tables = bacc.get_activation_tables

    def _patched_tables(arch):
        t = dict(orig_tables(arch))
        for name, fns in t.items():
            if name != "natural_log_exp_and_others":
                t[name] = {
                    f for f in fns
                    if f not in (mybir.ActivationFunctionType.Exp,
                                 mybir.ActivationFunctionType.Ln)
                }
        return t

    bacc.get_activation_tables = _patched_tables

    nc = bacc.Bacc(
        "TRN2", target_bir_lowering=False, debug=False, num_devices=N_CORES
    )

    xT_d = nc.dram_tensor("xT", [D, N], f16, kind="ExternalInput").ap()
    wq_d = nc.dram_tensor("wq", [D, CH], f16, kind="ExternalInput").ap()
    wk_d = nc.dram_tensor("wk", [D, CH], f16, kind="ExternalInput").ap()
    wv_d = nc.dram_tensor("wv", [D, CH], f16, kind="ExternalInput").ap()
    wo_d = nc.dram_tensor("wo", [CH, D], f16, kind="ExternalInput").ap()
    bq_d = nc.dram_tensor("bq", [CH, 1], f32, kind="ExternalInput").ap()
    bk_d = nc.dram_tensor("bk", [CH, 1], f32, kind="ExternalInput").ap()
    bv_d = nc.dram_tensor("bvb", [128, CH], f32, kind="ExternalInput").ap()
    bo_d = nc.dram_tensor("bob", [128, D], f32, kind="ExternalInput").ap()
    U_d = nc.dram_tensor("U", [128, 128], f16, kind="ExternalInput").ap()
    R_d = nc.dram_tensor("R", [128, 896], f16, kind="ExternalInput").ap()
    y_d = nc.dram_tensor("y", [N, D], f16, kind="ExternalOutput").ap()

    with tile.TileContext(nc) as tc, ExitStack() as ctx:
        const = ctx.enter_context(tc.tile_pool(name="const", bufs=1))
        actp = ctx.enter_context(tc.tile_pool(name="actp", bufs=1))
        work = ctx.enter_context(tc.tile_pool(name="work", bufs=3))
        ptp = ctx.enter_context(tc.tile_pool(name="ptp", bufs=5))
        normp = ctx.enter_context(tc.tile_pool(name="normp", bufs=2))
        ps_s = ctx.enter_context(tc.tile_pool(name="ps_s", bufs=2, space="PSUM"))
        ps_p = ctx.enter_context(tc.tile_pool(name="ps_p", bufs=4, space="PSUM"))

        # ---- constants / weights into SBUF ----
        # K-weights + first seq-chunk of x first so the K^T GEMM starts ASAP.
        wq = [const.tile([128, CH], f16, tag=f"wq{i}", name=f"wq{i}") for i in range(8)]
        wk = [const.tile([128, CH], f16, tag=f"wk{i}", name=f"wk{i}") for i in range(8)]
        wv = [const.tile([128, CH], f16, tag=f"wv{i}", name=f"wv{i}") for i in range(8)]
        xt = [[const.tile([128, 512], f16, tag=f"xt{i}_{sc}", name=f"xt{i}_{sc}")
               for sc in range(4)] for i in range(8)]
        # Round-robin input DMAs across engine queues so the 2D row-descriptor
        # processing runs in parallel; first-needed first. The first wave
        # (wk + first x chunk) additionally uses the vector/scalar queues,
        # which are compute-idle until the first GEMM finishes.
        first_engs = [nc.sync, nc.gpsimd, nc.scalar]
        engs = [nc.sync, nc.gpsimd]
        _ei = [0]

        def dma_first(dst, src):
            first_engs[_ei[0] % len(first_engs)].dma_start(dst, src)
            _ei[0] += 1

        def dma_in(dst, src):
            engs[_ei[0] % len(engs)].dma_start(dst, src)
            _ei[0] += 1

        # wave 1: just the ct=0 column slices of wk/wq + x chunk 0 + wv,
        # so the first KT/QT/V GEMMs can start after ~2.5MB instead of ~5MB.
        for i in range(8):
            dma_first(wk[i][:, 0:128], wk_d[i * 128 : (i + 1) * 128, 0:128])
            dma_first(xt[i][0][:], xT_d[i * 128 : (i + 1) * 128, 0:512])
        bq = [const.tile([128, 1], f32, tag=f"bq{j}", name=f"bq{j}") for j in range(4)]
        bk = [const.tile([128, 1], f32, tag=f"bk{j}", name=f"bk{j}") for j in range(4)]
        dma_first(bk[0][:], bk_d[0:128, :])
        dma_first(bq[0][:], bq_d[0:128, :])
        bv_t = const.tile([128, CH], f32, tag="bvb", name="bvb")
        dma_first(bv_t[:], bv_d[:])
        for i in range(8):
            dma_first(wq[i][:, 0:128], wq_d[i * 128 : (i + 1) * 128, 0:128])
            dma_first(wv[i][:], wv_d[i * 128 : (i + 1) * 128, :])
        # U/R are tiny (256KB) and attention(0)'s diagonal masking needs
        # them right after the first QKV groups -> load before wave 2.
        U_t = const.tile([128, 128], f16, tag="U", name="Ut")
        dma_first(U_t[:], U_d[:])
        R_t = const.tile([128, 896], f16, tag="R", name="Rt")
        dma_first(R_t[:], R_d[:])
        # wave 2: remainders and later chunks.
        for i in range(8):
            dma_in(wk[i][:, 128:CH], wk_d[i * 128 : (i + 1) * 128, 128:CH])
            dma_in(wq[i][:, 128:CH], wq_d[i * 128 : (i + 1) * 128, 128:CH])
        for j in range(1, 4):
            dma_in(bq[j][:], bq_d[j * 128 : (j + 1) * 128, :])
            dma_in(bk[j][:], bk_d[j * 128 : (j + 1) * 128, :])
        for sc in range(1, 4):
            for i in range(8):
                dma_in(xt[i][sc][:],
                       xT_d[i * 128 : (i + 1) * 128, sc * 512 : (sc + 1) * 512])
        wo = [const.tile([128, D], f16, tag=f"wo{j}", name=f"wo{j}") for j in range(4)]
        for j in range(4):
            dma_in(wo[j][:], wo_d[j * 128 : (j + 1) * 128, :])
        bo_t = const.tile([128, D], f32, tag="bob", name="bob")
        dma_in(bo_t[:], bo_d[:])

        # ---- persistent activations ----
        QT = [[actp.tile([128, 512], f16, tag=f"qt{ct}_{sc}", name=f"qt{ct}_{sc}") for sc in range(4)]
              for ct in range(4)]
        KT = [[actp.tile([128, 512], f16, tag=f"kt{ct}_{sc}", name=f"kt{ct}_{sc}") for sc in range(4)]
              for ct in range(4)]
        V = [actp.tile([128, 8 * 65], f16, tag=f"v{st}", name=f"v{st}") for st in range(16)]
        OTn = [[actp.tile([128, 512], f16, tag=f"otn{hp}_{qc}", name=f"otn{hp}_{qc}") for qc in range(4)]
               for hp in range(4)]

        # ---- PE filler quanta -------------------------------------------
        # The attention kt loop leaves the PE ~40% idle (waiting on ACT exp).
        # Those slots are filled by popping emission closures from a queue:
        # the previous chunk's out-projection groups and the next chunk's
        # QKV GEMM groups.
        filler = []

        def pop_filler():
            if filler:
                filler.pop(0)()

        def drain_filler():
            while filler:
                filler.pop(0)()

        def emit_outproj_group(qc, stl, oc):
            st = 4 * qc + stl
            sl = slice(stl * 128, (stl + 1) * 128)
            ocs = slice(oc * 512, (oc + 1) * 512)
            # hpp=3 first: the group's first matmul then depends on the LAST
            # head-pair's normalization, so the scheduler cannot hoist it
            # into the middle of the kt stream where its semaphore wait
            # would block the whole PE queue.
            yp = ps_p.tile([128, 512], f32, tag="p512", name="p512")
            for hpp in (3, 2, 1, 0):
                nc.tensor.matmul(yp[:], OTn[hpp][qc][:, sl],
                                 wo[hpp][:, ocs],
                                 start=(hpp == 3), stop=(hpp == 0),
                                 skip_group_check=True)
            ysb = work.tile([128, 512], f16, tag="ysb", name="ysb")
            nc.vector.scalar_tensor_tensor(ysb[:], yp[:], 1.0,
                                           bo_t[:, ocs], mult, add)
            nc.sync.dma_start(y_d[st * 128 : (st + 1) * 128, ocs], ysb[:])

        def queue_outproj(qc):
            for stl in range(4):
                for oc in range(2):
                    filler.append(
                        lambda qc=qc, stl=stl, oc=oc: emit_outproj_group(qc, stl, oc))

        def emit_kt_group(ct, sc):
            cs = slice(ct * 128, (ct + 1) * 128)
            p = ps_p.tile([128, 512], f32, tag="p512", name="p512")
            for d in range(8):
                nc.tensor.matmul(p[:], wk[d][:, cs], xt[d][sc][:],
                                 start=(d == 0), stop=(d == 7),
                                 skip_group_check=True)
            nc.vector.tensor_scalar_add(KT[ct][sc][:], p[:], bk[ct][:])

        def emit_qt_group(ct, sc):
            cs = slice(ct * 128, (ct + 1) * 128)
            p = ps_p.tile([128, 512], f32, tag="p512", name="p512")
            for d in range(8):
                nc.tensor.matmul(p[:], wq[d][:, cs], xt[d][sc][:],
                                 start=(d == 0), stop=(d == 7),
                                 skip_group_check=True)
            nc.vector.tensor_scalar_add(QT[ct][sc][:], p[:], bq[ct][:])

        def emit_v_group(stl, sc):
            st = 4 * sc + stl
            ts = slice(stl * 128, (stl + 1) * 128)
            p = ps_p.tile([128, 512], f32, tag="p512", name="p512")
            for d in range(8):
                nc.tensor.matmul(p[:], xt[d][sc][:, ts], wv[d][:, :],
                                 start=(d == 0), stop=(d == 7),
                                 skip_group_check=True)
            v3 = V[st][:].rearrange("p (h e) -> p h e", e=65)
            nc.vector.scalar_tensor_tensor(
                v3[:, :, 0:64],
                p[:].rearrange("p (h e) -> p h e", e=64),
                1.0,
                bv_t[:].rearrange("p (h e) -> p h e", e=64),
                mult, add,
            )
            nc.vector.memset(v3[:, :, 64:65], 1.0)

        def queue_qkv(sc):
            # K first (S-matmul stationary), then V, then Q.
            for ct in range(4):
                filler.append(lambda ct=ct, sc=sc: emit_kt_group(ct, sc))
            for stl in range(4):
                filler.append(lambda stl=stl, sc=sc: emit_v_group(stl, sc))
            for ct in range(4):
                filler.append(lambda ct=ct, sc=sc: emit_qt_group(ct, sc))

        # ---- attention software pipeline state (spans hp/chunk boundaries) --
        pend = []    # entries awaiting their AV matmuls
        flip = [0]

        def emit_norm(e):
            # softmax normalization: evacuate av PSUM to SBUF right away
            # (frees the PSUM pool), then normalize from SBUF off the
            # critical path. Row 64 of av = softmax denominator.
            hp, qc = e["hp"], e["qc"]
            avsb = normp.tile([65, 1024], f32, tag="avsb", name="avsb")
            nc.vector.tensor_copy(avsb[:, 0:512], e["av0"][:])
            nc.vector.tensor_copy(avsb[:, 512:1024], e["av1"][:])
            # 1/d = exp(-ln(d)) on ACT (both fns share one table set): a
            # [1,N] reciprocal on the DVE (3.3us, single lane) sits on the
            # OTn dependence chain and stalls the PE queue behind it.
            r = work.tile([1, 1024], f32, tag="r", name="r")
            lnt = work.tile([1, 1024], f32, tag="lnt", name="lnt")
            nc.scalar.activation(lnt[:], avsb[64:65, :], Ln)
            nc.scalar.activation(r[:], lnt[:], Exp, scale=-1.0)
            rb0 = work.tile([64, 512], f32, tag="rb", name="rb0")
            nc.gpsimd.partition_broadcast(rb0[:], r[0:1, 0:512], channels=64)
            nc.vector.tensor_mul(OTn[hp][qc][0:64, :], avsb[0:64, 0:512],
                                 rb0[:])
            rb1 = work.tile([64, 512], f32, tag="rb", name="rb1")
            nc.gpsimd.partition_broadcast(rb1[:], r[0:1, 512:1024], channels=64)
            nc.vector.tensor_mul(OTn[hp][qc][64:128, :], avsb[0:64, 512:1024],
                                 rb1[:])

        def flush_av():
            e = pend.pop(0)
            d = e["delta"]
            nc.tensor.matmul(
                e["av0"][:, d:512], V[e["kt"]][:, e["h0"] * 65 : e["h0"] * 65 + 65],
                e["pt"][:, d:512],
                start=e["first"], stop=e["last"], skip_group_check=True)
            nc.tensor.matmul(
                e["av1"][:, d:512], V[e["kt"]][:, e["h1"] * 65 : e["h1"] * 65 + 65],
                e["pt"][:, 512 + d : 1024],
                start=e["first"], stop=e["last"], skip_group_check=True)
            if e["last"]:
                emit_norm(e)

        # ---- chunk 0 QKV: only what hp0's attention needs runs up front
        # (KT/QT ct=0 + all V); the other head-pairs' K/Q projections are
        # queued as filler consumed during earlier head-pairs' kt loops.
        emit_kt_group(0, 0)
        emit_qt_group(0, 0)
        for stl in range(4):
            emit_v_group(stl, 0)

        # Chunk 3's attention is ACT(exp)-bound while chunks 0-2 are
        # PE-bound: shift chunk 3's first head-pair into chunk 2's emission
        # window (its QKV(3) inputs are produced by chunk 2's fillers well
        # before the appended item runs).
        # Cascade: each chunk absorbs the NEXT chunk's leading head-pair(s)
        # into its PE-bound window, pushing exp work forward so the final
        # (purely ACT-bound) chunk shrinks. All inputs of a shifted item are
        # produced by fillers popped earlier in the same chunk.
        plan = [
            [(0, 0), (0, 1), (0, 2), (0, 3), (1, 0)],
            [(1, 1), (1, 2), (1, 3), (2, 0)],
            [(2, 1), (2, 2), (2, 3), (3, 0), (3, 1)],
            [(3, 2), (3, 3)],
        ]
        for sc in range(4):
            # Queue this chunk's deferred PE work: previous chunk's
            # out-projection, then the NEXT chunk's QKV projections.
            if sc == 0:
                for ct in range(1, 4):
                    filler.append(lambda ct=ct: emit_kt_group(ct, 0))
                    filler.append(lambda ct=ct: emit_qt_group(ct, 0))
            # QKV(sc+1) first: the out-projection quanta need ALL of chunk
            # sc-1's OTn tiles, whose last normalization lands a few us into
            # this chunk -- popping them later avoids a PE stall. In the last
            # chunk (no QKV left, PE otherwise starved) outproj goes first.
            if sc < 3:
                queue_qkv(sc + 1)
            if sc > 0:
                queue_outproj(sc - 1)

            # Pace filler pops evenly over this chunk's kt slots (Bresenham)
            # instead of front-loading 1-per-kt: front-loading leaves the
            # late kt slots running at ACT rate with the PE starved. Chunk 0
            # keeps 1-per-kt: its first fillers are chunk-0 K/Q projections
            # that later head-pairs in the SAME chunk depend on.
            nslots = sum(4 * (q + 1) for q, _ in plan[sc])
            nfill0 = len(filler)
            state = {"slot": 0, "popped": 0}

            def paced_pop(sc=sc, nslots=nslots, nfill0=nfill0, state=state):
                state["slot"] += 1
                if sc == 0:
                    pop_filler()
                    state["popped"] += 1
                    return
                while (filler
                       and state["popped"] * nslots < state["slot"] * nfill0):
                    pop_filler()
                    state["popped"] += 1

            # attention: head pairs interleaved so the even head's K=64
            # matmuls (rows 0-63) and the odd head's (rows 64-127) run
            # concurrently in the PE array. The S->exp->AV software pipeline
            # (pend) runs FLAT across head-pair and chunk boundaries so the
            # ACT exp stream never drains at a boundary.
            for qc, hp in plan[sc]:
                nkt = 4 * (qc + 1)
                h0, h1 = 2 * hp, 2 * hp + 1
                av0 = ps_p.tile([65, 512], f32, tag="p512", name="av0")
                av1 = ps_p.tile([65, 512], f32, tag="p512", name="av1")

                for kt in range(nkt):
                    # merged S^T tile: cols 0-511 head h0, 512-1023 head h1.
                    # Diagonal-straddling blocks: cols < delta are fully
                    # masked -> not computed at all; the 128-wide strip
                    # [delta, delta+128) gets the -BIG ramp added.
                    diag = kt >= 4 * qc
                    delta = 128 * kt - 512 * qc if diag else 0
                    sp = ps_s.tile([128, 1024], f32, tag="s2", name="sp")
                    kcol = slice((kt % 4) * 128, (kt % 4) * 128 + 128)
                    nc.tensor.matmul(
                        sp[:, delta:512], KT[hp][kt // 4][0:64, kcol],
                        QT[hp][qc][0:64, delta:512],
                        start=True, stop=not diag, skip_group_check=True)
                    nc.tensor.matmul(
                        sp[:, 512 + delta : 1024], KT[hp][kt // 4][64:128, kcol],
                        QT[hp][qc][64:128, delta:512],
                        start=True, stop=not diag, skip_group_check=True)
                    if diag:
                        rsl = slice(384, 512)
                        nc.tensor.matmul(sp[:, delta : delta + 128], U_t[:],
                                         R_t[:, rsl],
                                         start=False, stop=True,
                                         skip_group_check=True)
                        nc.tensor.matmul(sp[:, 512 + delta : 512 + delta + 128],
                                         U_t[:], R_t[:, rsl],
                                         start=False, stop=True,
                                         skip_group_check=True)
                    pt = ptp.tile([128, 1024], f16, tag="pt", name="pt")
                    sp3 = sp[:].rearrange("p (h q) -> p h q", h=2)
                    pt3 = pt[:].rearrange("p (h q) -> p h q", h=2)
                    nc.scalar.activation(pt3[:, :, delta:512],
                                         sp3[:, :, delta:512], Exp, scale=SCALE)
                    pend.append(dict(kt=kt, pt=pt, delta=delta, av0=av0,
                                     av1=av1, h0=h0, h1=h1, hp=hp, qc=qc,
                                     first=(kt == 0), last=(kt == nkt - 1)))
                    paced_pop()
                    # AVs trail the exp stream by >=2 entries and are emitted
                    # in batches of 2: fewer S<->AV stationary switches means
                    # fewer un-hidden LDWEIGHTS on the PE.
                    flip[0] ^= 1
                    if flip[0] == 0:
                        while len(pend) > 2:
                            flush_av()
            drain_filler()
        while pend:
            flush_av()
        queue_outproj(3)
        drain_filler()

    nc.compile()
    bacc.get_activation_tables = orig_tables
    return nc


def _host_inputs(x, w_qkv, b_qkv, w_o, b_o):
    """Per-core input dicts implementing the sharding + layout prep."""
    U = np.zeros((128, 128), np.float16)
    for c in range(128):
        U[c, c:] = 1.0
    R = np.zeros((128, 896), np.float16)
    for c in range(128):
        R[c, : c + 384] = -BIG

    in_maps = []
    for c in range(N_CORES):
        b = c // 2
        hs = (c % 2) * HPC
        cols = slice(hs * DH, (hs + HPC) * DH)
        in_maps.append({
            "xT": np.ascontiguousarray(x[b].T).astype(np.float16),
            "wq": w_qkv[:, cols].astype(np.float16),
            "wk": w_qkv[:, D:][:, cols].astype(np.float16),
            "wv": w_qkv[:, 2 * D:][:, cols].astype(np.float16),
            "wo": w_o[hs * DH : (hs + HPC) * DH, :].astype(np.float16),
            "bq": b_qkv[cols].reshape(CH, 1).astype(np.float32),
            "bk": b_qkv[D:][cols].reshape(CH, 1).astype(np.float32),
            "bvb": np.tile(b_qkv[2 * D:][cols].astype(np.float32), (128, 1)),
            "bob": np.tile(b_o.astype(np.float32), (128, 1)),
            "U": U,
            "R": R,
        })
    return in_maps


def kernel(x, w_qkv, b_qkv, w_o, b_o):
    global _cached
    from concourse.bass_utils import run_bass_kernel_spmd

    x = np.asarray(x)
    w_qkv = np.asarray(w_qkv)
    b_qkv = np.asarray(b_qkv)
    w_o = np.asarray(w_o)
    b_o = np.asarray(b_o)

    if _cached is None:
        _cached = _build_program()
    nc = _cached

    in_maps = _host_inputs(x, w_qkv, b_qkv, w_o, b_o)
    res = run_bass_kernel_spmd(nc, in_maps, list(range(N_CORES)))

    out = np.empty((B, N, D), np.float32)
    for b in range(B):
        out[b] = (res.results[2 * b]["y"].astype(np.float32)
                  + res.results[2 * b + 1]["y"].astype(np.float32))
    return out
